# revision 4
# baseline (speedup 1.0000x reference)
"""Trainium2 Bass kernel for nn_DiarizationModel: 10 Adam iterations of
L1-basis fitting. T=50000 sharded over 8 cores; basis replicated.

Self-contained: hardcodes all shapes; host preps shards; device does the
10-iteration optimization; host transposes the tiny result back.

Main pass per 128-row T-chunk: PE computes psum = (B@A - E)^T via a bf16
identity-copy matmul of -E^T plus bf16 hi/lo B-split matmuls; vector does a
fused abs-reduce to per-column L1 sums. Argmax via Max8/MaxIndex + PE
transpose; candidate column fetched by dynamic-offset DMA; 8-core AllGather
of (max, j, column) blobs; winner row re-fetched by dynamic DMA; gradient,
Adam and shrink run replicated on all cores in fp32.
"""
import math
import numpy as np
import ml_dtypes

import concourse.bass as bass
import concourse.bacc as bacc
import concourse.mybir as mybir
import concourse.tile as tile
from concourse.bass_utils import run_bass_kernel_spmd

F32 = mybir.dt.float32
BF16 = mybir.dt.bfloat16
U32 = mybir.dt.uint32
I32 = mybir.dt.int32
AX = mybir.AxisListType
ALU = mybir.AluOpType
ACT = mybir.ActivationFunctionType

N_CORES = 8
D = 512
K = 16
T = 50000
SHARD = T // N_CORES        # 6250
NCH = 49                    # chunks of 128 T-rows per core
SHPAD = NCH * 128           # 6272
CW = D + K                  # 528 fused row width in gather tensor (negE | A)
FLATW = NCH * CW            # 25872
EW = NCH * D                # 25088 bf16 matmul tensor width

LAMBDA1 = 0.3366
LR = 0.1
CSH = LR * LAMBDA1
B1, B2, EPS = 0.9, 0.999, 1e-8
N_ITERS = 10
NEG_BIG = -1.0e30
N_BA_TERMS = 2              # 1: A_hi*B_hi ; 2: + A_hi*B_lo


def build_kernel():
    nc = bacc.Bacc(trn_type="TRN2", num_devices=N_CORES)

    get_d = nc.dram_tensor("get", [128, FLATW], F32, kind="ExternalInput")
    getbf_d = nc.dram_tensor("getbf", [128, EW], BF16, kind="ExternalInput")
    ahi_d = nc.dram_tensor("a_hi", [K, SHPAD], BF16, kind="ExternalInput")
    idb_d = nc.dram_tensor("identbf", [128, 128], BF16, kind="ExternalInput")
    idf_d = nc.dram_tensor("identf", [128, 128], F32, kind="ExternalInput")
    pt_d = nc.dram_tensor("ptinit", [K, D], F32, kind="ExternalInput")
    jb_d = nc.dram_tensor("jbase", [1, 1], F32, kind="ExternalInput")
    pm_d = nc.dram_tensor("padmask", [128, 1], F32, kind="ExternalInput")
    io_d = nc.dram_tensor("iotap", [1, 128], F32, kind="ExternalInput")

    obt_d = nc.dram_tensor("out_bt", [K, D], F32, kind="ExternalOutput")
    dbg_d = nc.dram_tensor("out_dbg", [1, 32], F32, kind="ExternalOutput")

    get_flat = get_d[:].rearrange("p f -> () (p f)")

    with tile.TileContext(nc) as tc:
        with tc.tile_pool(name="per", bufs=1) as per, \
             tc.tile_pool(name="scr", bufs=2) as scr, \
             tc.tile_pool(name="pmain", bufs=2, space="PSUM") as pmain, \
             tc.tile_pool(name="ptail", bufs=3, space="PSUM") as ptail, \
             tc.tile_pool(name="dram", bufs=2, space="DRAM") as dram:

            # ---- persistent tiles ----
            getbf = per.tile([128, EW], BF16)
            a_hi = per.tile([K, SHPAD], BF16)
            identb = per.tile([128, 128], BF16)
            identf = per.tile([128, 128], F32)
            padmask = per.tile([128, 1], F32)
            iotap = per.tile([1, 128], F32)
            jbase = per.tile([1, 1], F32)
            Ct = per.tile([128, 64], F32)
            PT = per.tile([K, D], F32)
            mT = per.tile([K, D], F32)
            vT = per.tile([K, D], F32)
            BT = per.tile([K, D], F32)
            BTh = per.tile([K, D], BF16)
            BTl = per.tile([K, D], BF16)
            smult = per.tile([K, D], F32)
            sgnP = per.tile([K, D], F32)
            mask_l1 = per.tile([K, 1], F32)
            dbg = per.tile([1, 32], F32)

            idf = identf[:]

            # ---- loads ----
            NSLAB = 7
            for s in range(NSLAB):
                w = EW // NSLAB
                nc.sync.dma_start(getbf[:, s * w:(s + 1) * w],
                                  getbf_d[:, s * w:(s + 1) * w])
            nc.sync.dma_start(a_hi[:], ahi_d[:])
            nc.sync.dma_start(identb[:], idb_d[:])
            nc.sync.dma_start(identf[:], idf_d[:])
            nc.sync.dma_start(padmask[:], pm_d[:])
            nc.sync.dma_start(iotap[:], io_d[:])
            nc.sync.dma_start(jbase[:], jb_d[:])
            nc.sync.dma_start(PT[:], pt_d[:])

            nc.vector.memset(mT[:], 0.0)
            nc.vector.memset(vT[:], 0.0)
            nc.vector.memset(Ct[:], NEG_BIG)
            nc.vector.memset(dbg[:], 0.0)
            nc.vector.tensor_copy(BT[:], PT[:])          # iter-1 B = P
            nc.scalar.copy(BTh[:], BT[:])
            nc.vector.scalar_tensor_tensor(BTl[:], BTh[:], -1.0, BT[:],
                                           op0=ALU.mult, op1=ALU.add)
            nc.scalar.sign(sgnP[:], PT[:])

            # PE warm-up touches: absorb slab DMA waits one at a time
            for s in range(NSLAB):
                wm = ptail.tile([1, 128], BF16, name="wm", tag="tail")
                nc.tensor.transpose(wm[:], getbf[:, s * (EW // NSLAB):
                                                 s * (EW // NSLAB) + 1],
                                    identb[:])
            wm2 = ptail.tile([1, K], BF16, name="wm2", tag="tail")
            nc.tensor.transpose(wm2[:], a_hi[:, 0:1], identb[0:K, 0:K])
            gsem = nc.alloc_semaphore("gsem")
            gcnt = 0

            for t in range(1, N_ITERS + 1):
                c2t = 1.0 - B2 ** t
                kt = LR / (1.0 - B1 ** t)

                # ---------- main pass: colsums of |B A - E| ----------
                for grp in range(25):
                    ng = 2 if grp < 24 else 1
                    ps = pmain.tile([128, 1024], F32, name="ps", tag="mainps")
                    for q in range(ng):
                        ch = grp * 2 + q
                        o = ps[:, q * 512:(q + 1) * 512]
                        nc.tensor.matmul(o, identb[:],
                                         getbf[:, ch * D:(ch + 1) * D],
                                         start=True, stop=False)
                        nc.tensor.matmul(o, a_hi[:, ch * 128:(ch + 1) * 128],
                                         BTh[:], start=False,
                                         stop=(N_BA_TERMS == 1))
                        if N_BA_TERMS >= 2:
                            nc.tensor.matmul(
                                o, a_hi[:, ch * 128:(ch + 1) * 128],
                                BTl[:], start=False, stop=True)
                    view = ps[:, 0:ng * 512].rearrange("p (n d) -> p n d", d=512)
                    nc.vector.tensor_reduce(Ct[:, grp * 2: grp * 2 + ng], view,
                                            axis=AX.X, op=ALU.add,
                                            apply_absolute_value=True)

                # mask pad rows of last chunk
                nc.vector.tensor_scalar(Ct[:, 48:49], Ct[:, 48:49],
                                        padmask[:], None, op0=ALU.add)

                # ---------- local argmax ----------
                m8 = scr.tile([128, 8], F32, name="m8", tag="m8")
                i8 = scr.tile([128, 8], U32, name="i8", tag="i8")
                nc.vector.max(m8[:], Ct[:])
                nc.vector.max_index(i8[:], m8[:], Ct[:])
                stk = scr.tile([128, 2], F32, name="stk", tag="stk")
                nc.vector.tensor_copy(stk[:, 0:1], m8[:, 0:1])
                nc.vector.tensor_copy(stk[:, 1:2], i8[:, 0:1])
                tpm = ptail.tile([1, 128], F32, name="tpm", tag="tail")
                tpi = ptail.tile([1, 128], F32, name="tpi", tag="tail")
                nc.tensor.transpose(tpm[:], stk[:, 0:1], idf)
                nc.tensor.transpose(tpi[:], stk[:, 1:2], idf)
                rmx = scr.tile([1, 128], F32, name="rmx", tag="rmx")
                rix = scr.tile([1, 128], F32, name="rix", tag="rix")
                nc.scalar.copy(rmx[:], tpm[:])
                nc.scalar.copy(rix[:], tpi[:])
                g8 = scr.tile([1, 8], F32, name="g8", tag="g8")
                gi8 = scr.tile([1, 8], U32, name="gi8", tag="gi8")
                nc.vector.max(g8[:], rmx[:])
                nc.vector.max_index(gi8[:], g8[:], rmx[:])
                ploc = scr.tile([1, 1], F32, name="ploc", tag="ploc")
                nc.vector.tensor_copy(ploc[:], gi8[:, 0:1])
                # cloc = rix[ploc]
                eqm = scr.tile([1, 128], F32, name="eqm", tag="eqm")
                nc.vector.tensor_scalar(eqm[:], iotap[:], ploc[:], None,
                                        op0=ALU.is_equal)
                nc.vector.tensor_tensor(out=eqm[:], in0=eqm[:], in1=rix[:],
                                        op=ALU.mult)
                cloc = scr.tile([1, 1], F32, name="cloc", tag="cloc")
                nc.vector.tensor_reduce(cloc[:], eqm[:], axis=AX.X, op=ALU.add)
                # jglob = jbase + cloc*128 + ploc ; off = ploc*FLATW + cloc*CW
                jg = scr.tile([1, 1], F32, name="jg", tag="jg")
                nc.vector.scalar_tensor_tensor(jg[:], cloc[:], 128.0, ploc[:],
                                               op0=ALU.mult, op1=ALU.add)
                nc.vector.tensor_tensor(out=jg[:], in0=jg[:], in1=jbase[:],
                                        op=ALU.add)
                offf = scr.tile([1, 1], F32, name="offf", tag="offf")
                nc.vector.tensor_scalar_mul(offf[:], cloc[:], float(CW))
                nc.vector.scalar_tensor_tensor(offf[:], ploc[:], float(FLATW),
                                               offf[:], op0=ALU.mult, op1=ALU.add)
                offi = scr.tile([1, 1], I32, name="offi", tag="offi")
                nc.vector.tensor_copy(offi[:], offf[:])

                # ---------- candidate blob + AllGather ----------
                blob = scr.tile([1, 536], F32, name="blob", tag="blob")
                nc.scalar.copy(blob[:, 0:1], g8[:, 0:1])
                nc.scalar.copy(blob[:, 1:2], jg[:])
                with tc.tile_critical():
                    offv = nc.gpsimd.value_load(offi[:])
                    gcnt += 16
                    nc.gpsimd.dma_start(
                        blob[:, 2:2 + CW],
                        get_flat[0:1, bass.ds(offv, CW)]).then_inc(gsem, 16)
                    nc.gpsimd.wait_ge(gsem, gcnt)
                agi = dram.tile([1, 536], F32, name="agi", tag="agi")
                ago = dram.tile([8, 536], F32, name="ago", tag="ago",
                                addr_space="Shared")
                nc.sync.dma_start(agi[:], blob[:])
                nc.gpsimd.collective_compute(
                    "AllGather", ALU.bypass,
                    replica_groups=[list(range(N_CORES))],
                    ins=[agi[:]], outs=[ago[:]])

                # ---------- winner ----------
                vals8 = scr.tile([1, 8], F32, name="vals8", tag="vals8")
                with nc.allow_non_contiguous_dma(reason="8-elem maxval gather"):
                    nc.sync.dma_start(vals8[:], ago[:, 0:1].transpose([1, 0]))
                w8 = scr.tile([1, 8], F32, name="w8", tag="w8")
                wi8 = scr.tile([1, 8], U32, name="wi8", tag="wi8")
                nc.vector.max(w8[:], vals8[:])
                nc.vector.max_index(wi8[:], w8[:], vals8[:])
                winner = scr.tile([1, 1 + CW], F32, name="winner", tag="winner")
                with tc.tile_critical():
                    wv = nc.gpsimd.value_load(wi8[0:1, 0:1])
                    gcnt += 16
                    nc.gpsimd.dma_start(
                        winner[:], ago[bass.ds(wv, 1), 1:2 + CW]).then_inc(gsem, 16)
                    nc.gpsimd.wait_ge(gsem, gcnt)

                # debug capture
                nc.scalar.copy(dbg[:, t - 1:t], winner[:, 0:1])
                nc.scalar.copy(dbg[:, 9 + t:10 + t], w8[:, 0:1])

                # ---------- gradient (fp32 exact path) ----------
                acT = ptail.tile([K, 1], F32, name="acT", tag="tail")
                nc.tensor.transpose(acT[:], winner[:, 1 + D:1 + D + K],
                                    idf[0:1, 0:1])
                acol = scr.tile([K, 1], F32, name="acol", tag="acol")
                nc.scalar.copy(acol[:], acT[:])
                ba = ptail.tile([1, D], F32, name="ba", tag="tail")
                nc.tensor.matmul(ba[:], acol[:], BT[:], start=True, stop=True)
                u = scr.tile([1, D], F32, name="u", tag="u")
                nc.vector.tensor_tensor(out=u[:], in0=winner[:, 1:1 + D],
                                        in1=ba[:], op=ALU.add)
                srow = scr.tile([1, D], F32, name="srow", tag="srow")
                nc.scalar.sign(srow[:], u[:])
                g1T = ptail.tile([K, D], F32, name="g1T", tag="tail")
                nc.tensor.matmul(g1T[:], winner[:, 1 + D:1 + D + K], srow[:],
                                 start=True, stop=True)

                # g2: k* = argmax colsum|B|  (sign(B)=sgnP)
                cb = scr.tile([K, 1], F32, name="cb", tag="cb")
                nc.vector.tensor_reduce(cb[:], BT[:], axis=AX.X,
                                        op=ALU.add, apply_absolute_value=True)
                cbT = ptail.tile([1, K], F32, name="cbT", tag="tail")
                nc.tensor.transpose(cbT[:], cb[:], idf[0:K, 0:K])
                rcb = scr.tile([1, K], F32, name="rcb", tag="rcb")
                nc.scalar.copy(rcb[:], cbT[:])
                cb8 = scr.tile([1, 8], F32, name="cb8", tag="cb8")
                nc.vector.max(cb8[:], rcb[:])
                kmr = scr.tile([1, K], F32, name="kmr", tag="kmr")
                nc.vector.tensor_scalar(kmr[:], rcb[:], cb8[:, 0:1], None,
                                        op0=ALU.is_ge)
                kmT = ptail.tile([K, 1], F32, name="kmT", tag="tail")
                nc.tensor.transpose(kmT[:], kmr[:], idf[0:1, 0:1])
                nc.scalar.mul(mask_l1[:], kmT[:], LAMBDA1)

                gT = scr.tile([K, D], F32, name="gT", tag="gT")
                nc.vector.scalar_tensor_tensor(gT[:], sgnP[:], mask_l1[:],
                                               g1T[:], op0=ALU.mult, op1=ALU.add)
                if t >= 2:
                    nc.vector.tensor_tensor(out=gT[:], in0=gT[:], in1=smult[:],
                                            op=ALU.mult)

                # ---------- Adam ----------
                sq = scr.tile([K, D], F32, name="sq", tag="sq")
                nc.scalar.activation(sq[:], gT[:], ACT.Square,
                                     scale=math.sqrt(1.0 - B2))
                gs = scr.tile([K, D], F32, name="gs", tag="gs")
                nc.scalar.mul(gs[:], gT[:], 1.0 - B1)
                nc.vector.scalar_tensor_tensor(vT[:], vT[:], B2, sq[:],
                                               op0=ALU.mult, op1=ALU.add)
                nc.vector.scalar_tensor_tensor(mT[:], mT[:], B1, gs[:],
                                               op0=ALU.mult, op1=ALU.add)
                dn = scr.tile([K, D], F32, name="dn", tag="dn")
                nc.scalar.activation(dn[:], vT[:], ACT.Sqrt, scale=1.0 / c2t)
                nc.vector.tensor_scalar_add(dn[:], dn[:], EPS)
                rcp = scr.tile([K, D], F32, name="rcp", tag="rcp")
                rscr = scr.tile([K, D], F32, name="rscr", tag="rscr")
                nc.vector.reciprocal_approx_accurate(rcp[:], dn[:], rscr[:])
                nc.vector.tensor_tensor(out=rcp[:], in0=mT[:], in1=rcp[:],
                                        op=ALU.mult)
                nc.vector.scalar_tensor_tensor(PT[:], rcp[:], -kt, PT[:],
                                               op0=ALU.mult, op1=ALU.add)

                # ---------- shrink / next-iteration B ----------
                t1 = scr.tile([K, D], F32, name="t1", tag="t1")
                nc.vector.tensor_scalar_sub(t1[:], PT[:], CSH)
                qq = scr.tile([K, D], F32, name="qq", tag="qq")
                nc.vector.tensor_tensor(out=qq[:], in0=PT[:], in1=t1[:],
                                        op=ALU.mult)
                if t < N_ITERS:
                    nc.scalar.sign(smult[:], qq[:])
                    nc.vector.tensor_tensor(out=BT[:], in0=smult[:],
                                            in1=t1[:], op=ALU.mult)
                    nc.scalar.copy(BTh[:], BT[:])
                    nc.vector.scalar_tensor_tensor(BTl[:], BTh[:], -1.0, BT[:],
                                                   op0=ALU.mult, op1=ALU.add)
                    nc.scalar.sign(sgnP[:], PT[:])
                else:
                    sf = scr.tile([K, D], F32, name="sf", tag="sf")
                    nc.scalar.sign(sf[:], qq[:])
                    btf = scr.tile([K, D], F32, name="btf", tag="btf")
                    nc.vector.tensor_tensor(out=btf[:], in0=sf[:], in1=t1[:],
                                            op=ALU.mult)
                    nc.sync.dma_start(obt_d[:], btf[:])
                    nc.sync.dma_start(dbg_d[:], dbg[:])

    nc.compile()
    return nc


_CACHE = {}


def _prep_inputs(embedding, basis_init, activation_init):
    E = np.ascontiguousarray(embedding, dtype=np.float32)
    A = np.ascontiguousarray(activation_init, dtype=np.float32)
    B0 = np.ascontiguousarray(basis_init, dtype=np.float32)
    ET = np.ascontiguousarray(E.T)              # (T, D)
    identf = np.eye(128, dtype=np.float32)
    identb = identf.astype(ml_dtypes.bfloat16)
    ptinit = np.ascontiguousarray(B0.T)         # (K, D)
    padmask = np.zeros((128, 1), np.float32)
    padmask[SHARD - 48 * 128:, :] = NEG_BIG
    iotap = np.arange(128, dtype=np.float32).reshape(1, 128)

    in_maps = []
    for c in range(N_CORES):
        lo = c * SHARD
        slabE = np.zeros((SHPAD, D), np.float32)
        slabE[:SHARD] = -ET[lo:lo + SHARD]
        slabA = np.zeros((SHPAD, K), np.float32)
        slabA[:SHARD] = A[:, lo:lo + SHARD].T
        fused = np.concatenate([slabE, slabA], axis=1)        # (SHPAD, CW)
        get = np.ascontiguousarray(
            fused.reshape(NCH, 128, CW).transpose(1, 0, 2).reshape(128, FLATW))
        getbf = np.ascontiguousarray(
            slabE.reshape(NCH, 128, D).transpose(1, 0, 2).reshape(128, EW)
        ).astype(ml_dtypes.bfloat16)
        ash = np.zeros((K, SHPAD), np.float32)
        ash[:, :SHARD] = A[:, lo:lo + SHARD]
        in_maps.append({
            "get": get,
            "getbf": getbf,
            "a_hi": ash.astype(ml_dtypes.bfloat16),
            "identbf": identb,
            "identf": identf,
            "ptinit": ptinit,
            "jbase": np.array([[float(lo)]], np.float32),
            "padmask": padmask,
            "iotap": iotap,
        })
    return in_maps


def kernel(embedding, basis_init, activation_init, k, _want_debug=False):
    if "nc" not in _CACHE:
        _CACHE["nc"] = build_kernel()
    nc = _CACHE["nc"]
    in_maps = _prep_inputs(embedding, basis_init, activation_init)
    res = run_bass_kernel_spmd(nc, in_maps, core_ids=list(range(N_CORES)))
    r0 = res.results[0]
    out_b = np.ascontiguousarray(r0["out_bt"].T)      # (D, K)
    out_a = np.asarray(activation_init, dtype=np.float32)
    _CACHE["last_res"] = res
    _CACHE["last_dbg"] = r0["out_dbg"]
    if _want_debug:
        return (out_b, out_a), r0["out_dbg"]
    return out_b, out_a


# revision 5
# speedup vs baseline: 1.2075x; 1.2075x over previous
"""Trainium2 Bass kernel for nn_DiarizationModel: 10 Adam iterations of
L1-basis fitting. T=50000 sharded over 8 cores; basis replicated.

Self-contained: hardcodes all shapes; host preps shards; device does the
10-iteration optimization; host transposes the tiny result back.

Main pass per 128-row T-chunk: PE computes psum = (B@A - E)^T via a bf16
identity-copy matmul of -E^T plus bf16 hi/lo B-split matmuls; vector does a
fused abs-reduce to per-column L1 sums. Argmax via Max8/MaxIndex + PE
transpose; candidate column fetched by dynamic-offset DMA; 8-core AllGather
of (max, j, column) blobs; winner row re-fetched by dynamic DMA; gradient,
Adam and shrink run replicated on all cores in fp32.
"""
import math
import numpy as np
import ml_dtypes

import concourse.bass as bass
import concourse.bacc as bacc
import concourse.mybir as mybir
import concourse.tile as tile
from concourse.bass_utils import run_bass_kernel_spmd

F32 = mybir.dt.float32
BF16 = mybir.dt.bfloat16
U32 = mybir.dt.uint32
I32 = mybir.dt.int32
AX = mybir.AxisListType
ALU = mybir.AluOpType
ACT = mybir.ActivationFunctionType

N_CORES = 8
D = 512
K = 16
T = 50000
SHARD = T // N_CORES        # 6250
NCH = 49                    # chunks of 128 T-rows per core
SHPAD = NCH * 128           # 6272
CW = D + K                  # 528 fused row width in gather tensor (negE | A)
FLATW = NCH * CW            # 25872
EW = NCH * D                # 25088 bf16 matmul tensor width

LAMBDA1 = 0.3366
LR = 0.1
CSH = LR * LAMBDA1
B1, B2, EPS = 0.9, 0.999, 1e-8
N_ITERS = 10
NEG_BIG = -1.0e30
N_BA_TERMS = 1              # 1: A_hi*B_hi ; 2: + A_hi*B_lo


def build_kernel():
    nc = bacc.Bacc(trn_type="TRN2", num_devices=N_CORES)

    get_d = nc.dram_tensor("get", [128, FLATW], F32, kind="ExternalInput")
    getbf_d = nc.dram_tensor("getbf", [128, EW], BF16, kind="ExternalInput")
    ahi_d = nc.dram_tensor("a_hi", [K, SHPAD], BF16, kind="ExternalInput")
    idf_d = nc.dram_tensor("identf", [128, 128], F32, kind="ExternalInput")
    pt_d = nc.dram_tensor("ptinit", [K, D], F32, kind="ExternalInput")
    jb_d = nc.dram_tensor("jbase", [1, 1], F32, kind="ExternalInput")
    pm_d = nc.dram_tensor("padmask", [128, 1], F32, kind="ExternalInput")
    io_d = nc.dram_tensor("iotap", [1, 128], F32, kind="ExternalInput")

    obt_d = nc.dram_tensor("out_bt", [K, D], F32, kind="ExternalOutput")
    dbg_d = nc.dram_tensor("out_dbg", [1, 32], F32, kind="ExternalOutput")

    get_flat = get_d[:].rearrange("p f -> () (p f)")

    with tile.TileContext(nc) as tc:
        with tc.tile_pool(name="per", bufs=1) as per, \
             tc.tile_pool(name="scr", bufs=2) as scr, \
             tc.tile_pool(name="pmain", bufs=2, space="PSUM") as pmain, \
             tc.tile_pool(name="ptail", bufs=3, space="PSUM") as ptail, \
             tc.tile_pool(name="dram", bufs=2, space="DRAM") as dram:

            # ---- persistent tiles ----
            getbf = per.tile([128, EW], BF16)
            a_hi = per.tile([K, SHPAD], BF16)
            identf = per.tile([128, 128], F32)
            padmask = per.tile([128, 1], F32)
            iotap = per.tile([1, 128], F32)
            jbase = per.tile([1, 1], F32)
            Ct = per.tile([128, 64], F32)
            PT = per.tile([K, D], F32)
            mT = per.tile([K, D], F32)
            vT = per.tile([K, D], F32)
            BT = per.tile([K, D], F32)
            BTh = per.tile([K, D], BF16)
            BTl = per.tile([K, D], BF16)
            smult = per.tile([K, D], F32)
            sgnP = per.tile([K, D], F32)
            mask_l1 = per.tile([K, 1], F32)
            dbg = per.tile([1, 32], F32)

            idf = identf[:]

            # ---- loads ----
            NSLAB = 7
            for s in range(NSLAB):
                w = EW // NSLAB
                nc.sync.dma_start(getbf[:, s * w:(s + 1) * w],
                                  getbf_d[:, s * w:(s + 1) * w])
            nc.sync.dma_start(a_hi[:], ahi_d[:])
            nc.sync.dma_start(identf[:], idf_d[:])
            nc.sync.dma_start(padmask[:], pm_d[:])
            nc.sync.dma_start(iotap[:], io_d[:])
            nc.sync.dma_start(jbase[:], jb_d[:])
            nc.sync.dma_start(PT[:], pt_d[:])

            nc.vector.memset(mT[:], 0.0)
            nc.vector.memset(vT[:], 0.0)
            nc.vector.memset(Ct[:], NEG_BIG)
            nc.vector.memset(dbg[:], 0.0)
            nc.vector.tensor_copy(BT[:], PT[:])          # iter-1 B = P
            nc.scalar.copy(BTh[:], BT[:])
            if N_BA_TERMS >= 2:
                nc.vector.scalar_tensor_tensor(BTl[:], BTh[:], -1.0, BT[:],
                                               op0=ALU.mult, op1=ALU.add)
            nc.scalar.sign(sgnP[:], PT[:])

            # PE warm-up touches: absorb slab DMA waits one at a time
            getbf_f32 = getbf[:].bitcast(F32)
            for s in range(NSLAB):
                wm = ptail.tile([1, 128], F32, name="wm", tag="tail")
                nc.tensor.transpose(wm[:], getbf_f32[:, s * (EW // NSLAB) // 2:
                                                     s * (EW // NSLAB) // 2 + 1],
                                    idf)
            gsem = nc.alloc_semaphore("gsem")
            gcnt = 0

            for t in range(1, N_ITERS + 1):
                c2t = 1.0 - B2 ** t
                kt = LR / (1.0 - B1 ** t)

                # ---------- main pass: colsums of |B A - E| ----------
                for grp in range(25):
                    ng = 2 if grp < 24 else 1
                    ps = pmain.tile([128, 1024], F32, name="ps", tag="mainps")
                    for q in range(ng):
                        ch = grp * 2 + q
                        o = ps[:, q * 512:(q + 1) * 512]
                        nc.tensor.matmul(o, a_hi[:, ch * 128:(ch + 1) * 128],
                                         BTh[:], start=True,
                                         stop=(N_BA_TERMS == 1))
                        if N_BA_TERMS >= 2:
                            nc.tensor.matmul(
                                o, a_hi[:, ch * 128:(ch + 1) * 128],
                                BTl[:], start=False, stop=True)
                    w = ng * 512
                    nc.vector.tensor_tensor(
                        out=ps[:, 0:w], in0=ps[:, 0:w],
                        in1=getbf[:, grp * 1024: grp * 1024 + w], op=ALU.add)
                    for q in range(ng):
                        ch = grp * 2 + q
                        o = ps[:, q * 512:(q + 1) * 512]
                        nc.scalar.activation(o, o, ACT.Abs,
                                             accum_out=Ct[:, ch:ch + 1])

                # mask pad rows of last chunk
                nc.vector.tensor_scalar(Ct[:, 48:49], Ct[:, 48:49],
                                        padmask[:], None, op0=ALU.add)

                # ---------- local argmax ----------
                m8 = scr.tile([128, 8], F32, name="m8", tag="m8")
                i8 = scr.tile([128, 8], U32, name="i8", tag="i8")
                nc.vector.max(m8[:], Ct[:])
                nc.vector.max_index(i8[:], m8[:], Ct[:])
                ixf = scr.tile([128, 1], F32, name="ixf", tag="ixf")
                nc.vector.tensor_copy(ixf[:], i8[:, 0:1])
                tpm = ptail.tile([1, 128], F32, name="tpm", tag="tail")
                tpi = ptail.tile([1, 128], F32, name="tpi", tag="tail")
                nc.tensor.transpose(tpm[:], m8[:, 0:1], idf)
                nc.tensor.transpose(tpi[:], ixf[:], idf)
                rmx = scr.tile([1, 128], F32, name="rmx", tag="rmx")
                rix = scr.tile([1, 128], F32, name="rix", tag="rix")
                nc.scalar.copy(rmx[:], tpm[:])
                nc.scalar.copy(rix[:], tpi[:])
                g8 = scr.tile([1, 8], F32, name="g8", tag="g8")
                gi8 = scr.tile([1, 8], U32, name="gi8", tag="gi8")
                nc.vector.max(g8[:], rmx[:])
                nc.vector.max_index(gi8[:], g8[:], rmx[:])
                ploc = scr.tile([1, 1], F32, name="ploc", tag="ploc")
                nc.vector.tensor_copy(ploc[:], gi8[:, 0:1])
                # cloc = rix[ploc]
                eqm = scr.tile([1, 128], F32, name="eqm", tag="eqm")
                nc.vector.tensor_scalar(eqm[:], iotap[:], ploc[:], None,
                                        op0=ALU.is_equal)
                nc.vector.tensor_tensor(out=eqm[:], in0=eqm[:], in1=rix[:],
                                        op=ALU.mult)
                cloc = scr.tile([1, 1], F32, name="cloc", tag="cloc")
                nc.vector.tensor_reduce(cloc[:], eqm[:], axis=AX.X, op=ALU.add)
                # jglob = jbase + cloc*128 + ploc ; off = ploc*FLATW + cloc*CW
                jg = scr.tile([1, 1], F32, name="jg", tag="jg")
                nc.vector.scalar_tensor_tensor(jg[:], cloc[:], 128.0, ploc[:],
                                               op0=ALU.mult, op1=ALU.add)
                nc.vector.tensor_tensor(out=jg[:], in0=jg[:], in1=jbase[:],
                                        op=ALU.add)
                offf = scr.tile([1, 1], F32, name="offf", tag="offf")
                nc.vector.tensor_scalar_mul(offf[:], cloc[:], float(CW))
                nc.vector.scalar_tensor_tensor(offf[:], ploc[:], float(FLATW),
                                               offf[:], op0=ALU.mult, op1=ALU.add)
                offi = scr.tile([1, 1], I32, name="offi", tag="offi")
                nc.vector.tensor_copy(offi[:], offf[:])

                # ---------- candidate blob + AllGather ----------
                blob = scr.tile([1, 536], F32, name="blob", tag="blob")
                nc.scalar.copy(blob[:, 0:1], g8[:, 0:1])
                nc.scalar.copy(blob[:, 1:2], jg[:])
                with tc.tile_critical():
                    offv = nc.gpsimd.value_load(offi[:])
                    gcnt += 16
                    nc.gpsimd.dma_start(
                        blob[:, 2:2 + CW],
                        get_flat[0:1, bass.ds(offv, CW)]).then_inc(gsem, 16)
                    nc.gpsimd.wait_ge(gsem, gcnt)
                agi = dram.tile([1, 536], F32, name="agi", tag="agi")
                ago = dram.tile([8, 536], F32, name="ago", tag="ago",
                                addr_space="Shared")
                nc.sync.dma_start(agi[:], blob[:])
                nc.gpsimd.collective_compute(
                    "AllGather", ALU.bypass,
                    replica_groups=[list(range(N_CORES))],
                    ins=[agi[:]], outs=[ago[:]])

                # ---------- winner ----------
                vals8 = scr.tile([1, 8], F32, name="vals8", tag="vals8")
                with nc.allow_non_contiguous_dma(reason="8-elem maxval gather"):
                    nc.sync.dma_start(vals8[:], ago[:, 0:1].transpose([1, 0]))
                w8 = scr.tile([1, 8], F32, name="w8", tag="w8")
                wi8 = scr.tile([1, 8], U32, name="wi8", tag="wi8")
                nc.vector.max(w8[:], vals8[:])
                nc.vector.max_index(wi8[:], w8[:], vals8[:])
                winner = scr.tile([1, 1 + CW], F32, name="winner", tag="winner")
                with tc.tile_critical():
                    wv = nc.gpsimd.value_load(wi8[0:1, 0:1])
                    gcnt += 16
                    nc.gpsimd.dma_start(
                        winner[:], ago[bass.ds(wv, 1), 1:2 + CW]).then_inc(gsem, 16)
                    nc.gpsimd.wait_ge(gsem, gcnt)

                # debug capture
                nc.scalar.copy(dbg[:, t - 1:t], winner[:, 0:1])
                nc.scalar.copy(dbg[:, 9 + t:10 + t], w8[:, 0:1])

                # ---------- gradient (fp32 exact path) ----------
                acT = ptail.tile([K, 1], F32, name="acT", tag="tail")
                nc.tensor.transpose(acT[:], winner[:, 1 + D:1 + D + K],
                                    idf[0:1, 0:1])
                acol = scr.tile([K, 1], F32, name="acol", tag="acol")
                nc.scalar.copy(acol[:], acT[:])
                ba = ptail.tile([1, D], F32, name="ba", tag="tail")
                nc.tensor.matmul(ba[:], acol[:], BT[:], start=True, stop=True)
                u = scr.tile([1, D], F32, name="u", tag="u")
                nc.vector.tensor_tensor(out=u[:], in0=winner[:, 1:1 + D],
                                        in1=ba[:], op=ALU.add)
                srow = scr.tile([1, D], F32, name="srow", tag="srow")
                nc.scalar.sign(srow[:], u[:])
                g1T = ptail.tile([K, D], F32, name="g1T", tag="tail")
                nc.tensor.matmul(g1T[:], winner[:, 1 + D:1 + D + K], srow[:],
                                 start=True, stop=True)

                # g2: k* = argmax colsum|B|  (sign(B)=sgnP)
                cb = scr.tile([K, 1], F32, name="cb", tag="cb")
                nc.vector.tensor_reduce(cb[:], BT[:], axis=AX.X,
                                        op=ALU.add, apply_absolute_value=True)
                cbT = ptail.tile([1, K], F32, name="cbT", tag="tail")
                nc.tensor.transpose(cbT[:], cb[:], idf[0:K, 0:K])
                rcb = scr.tile([1, K], F32, name="rcb", tag="rcb")
                nc.scalar.copy(rcb[:], cbT[:])
                cb8 = scr.tile([1, 8], F32, name="cb8", tag="cb8")
                nc.vector.max(cb8[:], rcb[:])
                kmr = scr.tile([1, K], F32, name="kmr", tag="kmr")
                nc.vector.tensor_scalar(kmr[:], rcb[:], cb8[:, 0:1], None,
                                        op0=ALU.is_ge)
                kmT = ptail.tile([K, 1], F32, name="kmT", tag="tail")
                nc.tensor.transpose(kmT[:], kmr[:], idf[0:1, 0:1])
                nc.scalar.mul(mask_l1[:], kmT[:], LAMBDA1)

                gT = scr.tile([K, D], F32, name="gT", tag="gT")
                nc.vector.scalar_tensor_tensor(gT[:], sgnP[:], mask_l1[:],
                                               g1T[:], op0=ALU.mult, op1=ALU.add)
                if t >= 2:
                    nc.vector.tensor_tensor(out=gT[:], in0=gT[:], in1=smult[:],
                                            op=ALU.mult)

                # ---------- Adam ----------
                sq = scr.tile([K, D], F32, name="sq", tag="sq")
                nc.scalar.activation(sq[:], gT[:], ACT.Square,
                                     scale=math.sqrt(1.0 - B2))
                gs = scr.tile([K, D], F32, name="gs", tag="gs")
                nc.scalar.mul(gs[:], gT[:], 1.0 - B1)
                nc.vector.scalar_tensor_tensor(vT[:], vT[:], B2, sq[:],
                                               op0=ALU.mult, op1=ALU.add)
                nc.vector.scalar_tensor_tensor(mT[:], mT[:], B1, gs[:],
                                               op0=ALU.mult, op1=ALU.add)
                dn = scr.tile([K, D], F32, name="dn", tag="dn")
                nc.scalar.activation(dn[:], vT[:], ACT.Sqrt, scale=1.0 / c2t)
                nc.vector.tensor_scalar_add(dn[:], dn[:], EPS)
                rcp = scr.tile([K, D], F32, name="rcp", tag="rcp")
                rscr = scr.tile([K, D], F32, name="rscr", tag="rscr")
                nc.vector.reciprocal_approx_accurate(rcp[:], dn[:], rscr[:])
                nc.vector.tensor_tensor(out=rcp[:], in0=mT[:], in1=rcp[:],
                                        op=ALU.mult)
                nc.vector.scalar_tensor_tensor(PT[:], rcp[:], -kt, PT[:],
                                               op0=ALU.mult, op1=ALU.add)

                # ---------- shrink / next-iteration B ----------
                t1 = scr.tile([K, D], F32, name="t1", tag="t1")
                nc.vector.tensor_scalar_sub(t1[:], PT[:], CSH)
                qq = scr.tile([K, D], F32, name="qq", tag="qq")
                nc.vector.tensor_tensor(out=qq[:], in0=PT[:], in1=t1[:],
                                        op=ALU.mult)
                if t < N_ITERS:
                    nc.scalar.sign(smult[:], qq[:])
                    nc.vector.tensor_tensor(out=BT[:], in0=smult[:],
                                            in1=t1[:], op=ALU.mult)
                    nc.scalar.copy(BTh[:], BT[:])
                    if N_BA_TERMS >= 2:
                        nc.vector.scalar_tensor_tensor(
                            BTl[:], BTh[:], -1.0, BT[:],
                            op0=ALU.mult, op1=ALU.add)
                    nc.scalar.sign(sgnP[:], PT[:])
                else:
                    sf = scr.tile([K, D], F32, name="sf", tag="sf")
                    nc.scalar.sign(sf[:], qq[:])
                    btf = scr.tile([K, D], F32, name="btf", tag="btf")
                    nc.vector.tensor_tensor(out=btf[:], in0=sf[:], in1=t1[:],
                                            op=ALU.mult)
                    nc.sync.dma_start(obt_d[:], btf[:])
                    nc.sync.dma_start(dbg_d[:], dbg[:])

    nc.compile()
    return nc


_CACHE = {}


def _prep_inputs(embedding, basis_init, activation_init):
    E = np.ascontiguousarray(embedding, dtype=np.float32)
    A = np.ascontiguousarray(activation_init, dtype=np.float32)
    B0 = np.ascontiguousarray(basis_init, dtype=np.float32)
    ET = np.ascontiguousarray(E.T)              # (T, D)
    identf = np.eye(128, dtype=np.float32)
    ptinit = np.ascontiguousarray(B0.T)         # (K, D)
    padmask = np.zeros((128, 1), np.float32)
    padmask[SHARD - 48 * 128:, :] = NEG_BIG
    iotap = np.arange(128, dtype=np.float32).reshape(1, 128)

    in_maps = []
    for c in range(N_CORES):
        lo = c * SHARD
        slabE = np.zeros((SHPAD, D), np.float32)
        slabE[:SHARD] = -ET[lo:lo + SHARD]
        slabA = np.zeros((SHPAD, K), np.float32)
        slabA[:SHARD] = A[:, lo:lo + SHARD].T
        fused = np.concatenate([slabE, slabA], axis=1)        # (SHPAD, CW)
        get = np.ascontiguousarray(
            fused.reshape(NCH, 128, CW).transpose(1, 0, 2).reshape(128, FLATW))
        getbf = np.ascontiguousarray(
            slabE.reshape(NCH, 128, D).transpose(1, 0, 2).reshape(128, EW)
        ).astype(ml_dtypes.bfloat16)
        ash = np.zeros((K, SHPAD), np.float32)
        ash[:, :SHARD] = A[:, lo:lo + SHARD]
        in_maps.append({
            "get": get,
            "getbf": getbf,
            "a_hi": ash.astype(ml_dtypes.bfloat16),
            "identf": identf,
            "ptinit": ptinit,
            "jbase": np.array([[float(lo)]], np.float32),
            "padmask": padmask,
            "iotap": iotap,
        })
    return in_maps


def kernel(embedding, basis_init, activation_init, k, _want_debug=False):
    if "nc" not in _CACHE:
        _CACHE["nc"] = build_kernel()
    nc = _CACHE["nc"]
    in_maps = _prep_inputs(embedding, basis_init, activation_init)
    res = run_bass_kernel_spmd(nc, in_maps, core_ids=list(range(N_CORES)))
    r0 = res.results[0]
    out_b = np.ascontiguousarray(r0["out_bt"].T)      # (D, K)
    out_a = np.asarray(activation_init, dtype=np.float32)
    _CACHE["last_res"] = res
    _CACHE["last_dbg"] = r0["out_dbg"]
    if _want_debug:
        return (out_b, out_a), r0["out_dbg"]
    return out_b, out_a


# revision 6
# speedup vs baseline: 1.2288x; 1.0177x over previous
"""Trainium2 Bass kernel for nn_DiarizationModel: 10 Adam iterations of
L1-basis fitting. T=50000 sharded over 8 cores; basis replicated.

Self-contained: hardcodes all shapes; host preps shards; device does the
10-iteration optimization; host transposes the tiny result back.

Main pass per 128-row T-chunk: PE computes psum = (B@A - E)^T via a bf16
identity-copy matmul of -E^T plus bf16 hi/lo B-split matmuls; vector does a
fused abs-reduce to per-column L1 sums. Argmax via Max8/MaxIndex + PE
transpose; candidate column fetched by dynamic-offset DMA; 8-core AllGather
of (max, j, column) blobs; winner row re-fetched by dynamic DMA; gradient,
Adam and shrink run replicated on all cores in fp32.
"""
import math
import numpy as np
import ml_dtypes

import concourse.bass as bass
import concourse.bacc as bacc
import concourse.mybir as mybir
import concourse.tile as tile
from concourse.bass_utils import run_bass_kernel_spmd

F32 = mybir.dt.float32
BF16 = mybir.dt.bfloat16
U32 = mybir.dt.uint32
I32 = mybir.dt.int32
AX = mybir.AxisListType
ALU = mybir.AluOpType
ACT = mybir.ActivationFunctionType

N_CORES = 8
D = 512
K = 16
T = 50000
SHARD = T // N_CORES        # 6250
NCH = 49                    # chunks of 128 T-rows per core
SHPAD = NCH * 128           # 6272
CW = D + K                  # 528 fused row width in gather tensor (negE | A)
FLATW = NCH * CW            # 25872
EW = NCH * D                # 25088 bf16 matmul tensor width

LAMBDA1 = 0.3366
LR = 0.1
CSH = LR * LAMBDA1
B1, B2, EPS = 0.9, 0.999, 1e-8
N_ITERS = 10
NEG_BIG = -1.0e30
N_BA_TERMS = 1              # 1: A_hi*B_hi ; 2: + A_hi*B_lo


def build_kernel():
    nc = bacc.Bacc(trn_type="TRN2", num_devices=N_CORES)

    get_d = nc.dram_tensor("get", [128, FLATW], F32, kind="ExternalInput")
    getbf_d = nc.dram_tensor("getbf", [128, EW], BF16, kind="ExternalInput")
    ahi_d = nc.dram_tensor("a_hi", [K, SHPAD], BF16, kind="ExternalInput")
    idb_d = nc.dram_tensor("identbf", [128, 128], BF16, kind="ExternalInput")
    idf_d = nc.dram_tensor("identf", [128, 128], F32, kind="ExternalInput")
    pt_d = nc.dram_tensor("ptinit", [K, D], F32, kind="ExternalInput")
    jb_d = nc.dram_tensor("jbase", [1, 1], F32, kind="ExternalInput")
    pm_d = nc.dram_tensor("padmask", [128, 1], F32, kind="ExternalInput")
    io_d = nc.dram_tensor("iotap", [1, 128], F32, kind="ExternalInput")

    obt_d = nc.dram_tensor("out_bt", [K, D], F32, kind="ExternalOutput")
    dbg_d = nc.dram_tensor("out_dbg", [1, 32], F32, kind="ExternalOutput")

    get_flat = get_d[:].rearrange("p f -> () (p f)")

    with tile.TileContext(nc) as tc:
        with tc.tile_pool(name="per", bufs=1) as per, \
             tc.tile_pool(name="scr", bufs=2) as scr, \
             tc.tile_pool(name="pmain", bufs=2, space="PSUM") as pmain, \
             tc.tile_pool(name="ptail", bufs=3, space="PSUM") as ptail, \
             tc.tile_pool(name="dram", bufs=2, space="DRAM") as dram:

            # ---- persistent tiles ----
            getbf = per.tile([128, EW], BF16)
            a_hi = per.tile([K, SHPAD], BF16)
            identb = per.tile([128, 128], BF16)
            identf = per.tile([128, 128], F32)
            padmask = per.tile([128, 1], F32)
            iotap = per.tile([1, 128], F32)
            jbase = per.tile([1, 1], F32)
            Ct = per.tile([128, 64], F32)
            PT = per.tile([K, D], F32)
            mT = per.tile([K, D], F32)
            vT = per.tile([K, D], F32)
            BT = per.tile([K, D], F32)
            BTh = per.tile([K, D], BF16)
            BTl = per.tile([K, D], BF16)
            smult = per.tile([K, D], F32)
            sgnP = per.tile([K, D], F32)
            mask_l1 = per.tile([K, 1], F32)
            dbg = per.tile([1, 32], F32)

            idf = identf[:]

            # ---- loads ----
            NSLAB = 7
            for s in range(NSLAB):
                w = EW // NSLAB
                nc.sync.dma_start(getbf[:, s * w:(s + 1) * w],
                                  getbf_d[:, s * w:(s + 1) * w])
            nc.sync.dma_start(a_hi[:], ahi_d[:])
            nc.sync.dma_start(identb[:], idb_d[:])
            nc.sync.dma_start(identf[:], idf_d[:])
            nc.sync.dma_start(padmask[:], pm_d[:])
            nc.sync.dma_start(iotap[:], io_d[:])
            nc.sync.dma_start(jbase[:], jb_d[:])
            nc.sync.dma_start(PT[:], pt_d[:])

            nc.vector.memset(mT[:], 0.0)
            nc.vector.memset(vT[:], 0.0)
            nc.vector.memset(Ct[:], NEG_BIG)
            nc.vector.memset(dbg[:], 0.0)
            nc.vector.tensor_copy(BT[:], PT[:])          # iter-1 B = P
            nc.scalar.copy(BTh[:], BT[:])
            if N_BA_TERMS >= 2:
                nc.vector.scalar_tensor_tensor(BTl[:], BTh[:], -1.0, BT[:],
                                               op0=ALU.mult, op1=ALU.add)
            nc.scalar.sign(sgnP[:], PT[:])

            # PE warm-up touches: absorb slab DMA waits one at a time
            for s in range(NSLAB):
                wm = ptail.tile([1, 128], BF16, name="wm", tag="tail")
                nc.tensor.transpose(wm[:], getbf[:, s * (EW // NSLAB):
                                                 s * (EW // NSLAB) + 1],
                                    identb[:])
            gsem = nc.alloc_semaphore("gsem")
            gcnt = 0

            for t in range(1, N_ITERS + 1):
                c2t = 1.0 - B2 ** t
                kt = LR / (1.0 - B1 ** t)

                # ---------- main pass: colsums of |B A - E| ----------
                for grp in range(25):
                    ng = 2 if grp < 24 else 1
                    ps = pmain.tile([128, 1024], F32, name="ps", tag="mainps")
                    for q in range(ng):
                        ch = grp * 2 + q
                        o = ps[:, q * 512:(q + 1) * 512]
                        nc.tensor.matmul(o, identb[:],
                                         getbf[:, ch * D:(ch + 1) * D],
                                         start=True, stop=False)
                        nc.tensor.matmul(o, a_hi[:, ch * 128:(ch + 1) * 128],
                                         BTh[:], start=False,
                                         stop=(N_BA_TERMS == 1))
                        if N_BA_TERMS >= 2:
                            nc.tensor.matmul(
                                o, a_hi[:, ch * 128:(ch + 1) * 128],
                                BTl[:], start=False, stop=True)
                    view = ps[:, 0:ng * 512].rearrange("p (n d) -> p n d", d=512)
                    nc.vector.tensor_reduce(Ct[:, grp * 2: grp * 2 + ng], view,
                                            axis=AX.X, op=ALU.add,
                                            apply_absolute_value=True)

                # mask pad rows of last chunk
                nc.vector.tensor_scalar(Ct[:, 48:49], Ct[:, 48:49],
                                        padmask[:], None, op0=ALU.add)

                # ---------- local argmax ----------
                m8 = scr.tile([128, 8], F32, name="m8", tag="m8")
                i8 = scr.tile([128, 8], U32, name="i8", tag="i8")
                nc.vector.max(m8[:], Ct[:])
                nc.vector.max_index(i8[:], m8[:], Ct[:])
                ixf = scr.tile([128, 1], F32, name="ixf", tag="ixf")
                nc.vector.tensor_copy(ixf[:], i8[:, 0:1])
                tpm = ptail.tile([1, 128], F32, name="tpm", tag="tail")
                tpi = ptail.tile([1, 128], F32, name="tpi", tag="tail")
                nc.tensor.transpose(tpm[:], m8[:, 0:1], idf)
                nc.tensor.transpose(tpi[:], ixf[:], idf)
                rmx = scr.tile([1, 128], F32, name="rmx", tag="rmx")
                rix = scr.tile([1, 128], F32, name="rix", tag="rix")
                nc.scalar.copy(rmx[:], tpm[:])
                nc.scalar.copy(rix[:], tpi[:])
                g8 = scr.tile([1, 8], F32, name="g8", tag="g8")
                gi8 = scr.tile([1, 8], U32, name="gi8", tag="gi8")
                nc.vector.max(g8[:], rmx[:])
                nc.vector.max_index(gi8[:], g8[:], rmx[:])
                ploc = scr.tile([1, 1], F32, name="ploc", tag="ploc")
                nc.vector.tensor_copy(ploc[:], gi8[:, 0:1])
                # cloc = rix[ploc]
                eqm = scr.tile([1, 128], F32, name="eqm", tag="eqm")
                nc.vector.tensor_scalar(eqm[:], iotap[:], ploc[:], None,
                                        op0=ALU.is_equal)
                nc.vector.tensor_tensor(out=eqm[:], in0=eqm[:], in1=rix[:],
                                        op=ALU.mult)
                cloc = scr.tile([1, 1], F32, name="cloc", tag="cloc")
                nc.vector.tensor_reduce(cloc[:], eqm[:], axis=AX.X, op=ALU.add)
                # jglob = jbase + cloc*128 + ploc ; off = ploc*FLATW + cloc*CW
                jg = scr.tile([1, 1], F32, name="jg", tag="jg")
                nc.vector.scalar_tensor_tensor(jg[:], cloc[:], 128.0, ploc[:],
                                               op0=ALU.mult, op1=ALU.add)
                nc.vector.tensor_tensor(out=jg[:], in0=jg[:], in1=jbase[:],
                                        op=ALU.add)
                offf = scr.tile([1, 1], F32, name="offf", tag="offf")
                nc.vector.tensor_scalar_mul(offf[:], cloc[:], float(CW))
                nc.vector.scalar_tensor_tensor(offf[:], ploc[:], float(FLATW),
                                               offf[:], op0=ALU.mult, op1=ALU.add)
                offi = scr.tile([1, 1], I32, name="offi", tag="offi")
                nc.vector.tensor_copy(offi[:], offf[:])

                # ---------- candidate blob + AllGather ----------
                blob = scr.tile([1, 536], F32, name="blob", tag="blob")
                nc.scalar.copy(blob[:, 0:1], g8[:, 0:1])
                nc.scalar.copy(blob[:, 1:2], jg[:])
                with tc.tile_critical():
                    offv = nc.gpsimd.value_load(offi[:])
                    gcnt += 16
                    nc.gpsimd.dma_start(
                        blob[:, 2:2 + CW],
                        get_flat[0:1, bass.ds(offv, CW)]).then_inc(gsem, 16)
                    nc.gpsimd.wait_ge(gsem, gcnt)
                agi = dram.tile([1, 536], F32, name="agi", tag="agi")
                ago = dram.tile([8, 536], F32, name="ago", tag="ago",
                                addr_space="Shared")
                nc.sync.dma_start(agi[:], blob[:])
                nc.gpsimd.collective_compute(
                    "AllGather", ALU.bypass,
                    replica_groups=[list(range(N_CORES))],
                    ins=[agi[:]], outs=[ago[:]])

                # ---------- winner ----------
                vals8 = scr.tile([1, 8], F32, name="vals8", tag="vals8")
                with nc.allow_non_contiguous_dma(reason="8-elem maxval gather"):
                    nc.sync.dma_start(vals8[:], ago[:, 0:1].transpose([1, 0]))
                w8 = scr.tile([1, 8], F32, name="w8", tag="w8")
                wi8 = scr.tile([1, 8], U32, name="wi8", tag="wi8")
                nc.vector.max(w8[:], vals8[:])
                nc.vector.max_index(wi8[:], w8[:], vals8[:])
                winner = scr.tile([1, 1 + CW], F32, name="winner", tag="winner")
                with tc.tile_critical():
                    wv = nc.gpsimd.value_load(wi8[0:1, 0:1])
                    gcnt += 16
                    nc.gpsimd.dma_start(
                        winner[:], ago[bass.ds(wv, 1), 1:2 + CW]).then_inc(gsem, 16)
                    nc.gpsimd.wait_ge(gsem, gcnt)

                # debug capture
                nc.scalar.copy(dbg[:, t - 1:t], winner[:, 0:1])
                nc.scalar.copy(dbg[:, 9 + t:10 + t], w8[:, 0:1])

                # ---------- gradient (fp32 exact path) ----------
                acT = ptail.tile([K, 1], F32, name="acT", tag="tail")
                nc.tensor.transpose(acT[:], winner[:, 1 + D:1 + D + K],
                                    idf[0:1, 0:1])
                acol = scr.tile([K, 1], F32, name="acol", tag="acol")
                nc.scalar.copy(acol[:], acT[:])
                ba = ptail.tile([1, D], F32, name="ba", tag="tail")
                nc.tensor.matmul(ba[:], acol[:], BT[:], start=True, stop=True)
                u = scr.tile([1, D], F32, name="u", tag="u")
                nc.vector.tensor_tensor(out=u[:], in0=winner[:, 1:1 + D],
                                        in1=ba[:], op=ALU.add)
                srow = scr.tile([1, D], F32, name="srow", tag="srow")
                nc.scalar.sign(srow[:], u[:])
                g1T = ptail.tile([K, D], F32, name="g1T", tag="tail")
                nc.tensor.matmul(g1T[:], winner[:, 1 + D:1 + D + K], srow[:],
                                 start=True, stop=True)

                # g2: k* = argmax colsum|B|  (sign(B)=sgnP)
                cb = scr.tile([K, 1], F32, name="cb", tag="cb")
                nc.vector.tensor_reduce(cb[:], BT[:], axis=AX.X,
                                        op=ALU.add, apply_absolute_value=True)
                cbT = ptail.tile([1, K], F32, name="cbT", tag="tail")
                nc.tensor.transpose(cbT[:], cb[:], idf[0:K, 0:K])
                rcb = scr.tile([1, K], F32, name="rcb", tag="rcb")
                nc.scalar.copy(rcb[:], cbT[:])
                cb8 = scr.tile([1, 8], F32, name="cb8", tag="cb8")
                nc.vector.max(cb8[:], rcb[:])
                kmr = scr.tile([1, K], F32, name="kmr", tag="kmr")
                nc.vector.tensor_scalar(kmr[:], rcb[:], cb8[:, 0:1], None,
                                        op0=ALU.is_ge)
                kmT = ptail.tile([K, 1], F32, name="kmT", tag="tail")
                nc.tensor.transpose(kmT[:], kmr[:], idf[0:1, 0:1])
                nc.scalar.mul(mask_l1[:], kmT[:], LAMBDA1)

                gT = scr.tile([K, D], F32, name="gT", tag="gT")
                nc.vector.scalar_tensor_tensor(gT[:], sgnP[:], mask_l1[:],
                                               g1T[:], op0=ALU.mult, op1=ALU.add)
                if t >= 2:
                    nc.vector.tensor_tensor(out=gT[:], in0=gT[:], in1=smult[:],
                                            op=ALU.mult)

                # ---------- Adam ----------
                sq = scr.tile([K, D], F32, name="sq", tag="sq")
                nc.scalar.activation(sq[:], gT[:], ACT.Square,
                                     scale=math.sqrt(1.0 - B2))
                gs = scr.tile([K, D], F32, name="gs", tag="gs")
                nc.scalar.mul(gs[:], gT[:], 1.0 - B1)
                nc.vector.scalar_tensor_tensor(vT[:], vT[:], B2, sq[:],
                                               op0=ALU.mult, op1=ALU.add)
                nc.vector.scalar_tensor_tensor(mT[:], mT[:], B1, gs[:],
                                               op0=ALU.mult, op1=ALU.add)
                dn = scr.tile([K, D], F32, name="dn", tag="dn")
                nc.scalar.activation(dn[:], vT[:], ACT.Sqrt, scale=1.0 / c2t)
                nc.vector.tensor_scalar_add(dn[:], dn[:], EPS)
                rcp = scr.tile([K, D], F32, name="rcp", tag="rcp")
                rscr = scr.tile([K, D], F32, name="rscr", tag="rscr")
                nc.vector.reciprocal_approx_accurate(rcp[:], dn[:], rscr[:])
                nc.vector.tensor_tensor(out=rcp[:], in0=mT[:], in1=rcp[:],
                                        op=ALU.mult)
                nc.vector.scalar_tensor_tensor(PT[:], rcp[:], -kt, PT[:],
                                               op0=ALU.mult, op1=ALU.add)

                # ---------- shrink / next-iteration B ----------
                t1 = scr.tile([K, D], F32, name="t1", tag="t1")
                nc.vector.tensor_scalar_sub(t1[:], PT[:], CSH)
                qq = scr.tile([K, D], F32, name="qq", tag="qq")
                nc.vector.tensor_tensor(out=qq[:], in0=PT[:], in1=t1[:],
                                        op=ALU.mult)
                if t < N_ITERS:
                    nc.scalar.sign(smult[:], qq[:])
                    nc.vector.tensor_tensor(out=BT[:], in0=smult[:],
                                            in1=t1[:], op=ALU.mult)
                    nc.scalar.copy(BTh[:], BT[:])
                    if N_BA_TERMS >= 2:
                        nc.vector.scalar_tensor_tensor(
                            BTl[:], BTh[:], -1.0, BT[:],
                            op0=ALU.mult, op1=ALU.add)
                    nc.scalar.sign(sgnP[:], PT[:])
                else:
                    sf = scr.tile([K, D], F32, name="sf", tag="sf")
                    nc.scalar.sign(sf[:], qq[:])
                    btf = scr.tile([K, D], F32, name="btf", tag="btf")
                    nc.vector.tensor_tensor(out=btf[:], in0=sf[:], in1=t1[:],
                                            op=ALU.mult)
                    nc.sync.dma_start(obt_d[:], btf[:])
                    nc.sync.dma_start(dbg_d[:], dbg[:])

    nc.compile()
    return nc


_CACHE = {}


def _prep_inputs(embedding, basis_init, activation_init):
    E = np.ascontiguousarray(embedding, dtype=np.float32)
    A = np.ascontiguousarray(activation_init, dtype=np.float32)
    B0 = np.ascontiguousarray(basis_init, dtype=np.float32)
    ET = np.ascontiguousarray(E.T)              # (T, D)
    identf = np.eye(128, dtype=np.float32)
    identb = identf.astype(ml_dtypes.bfloat16)
    ptinit = np.ascontiguousarray(B0.T)         # (K, D)
    padmask = np.zeros((128, 1), np.float32)
    padmask[SHARD - 48 * 128:, :] = NEG_BIG
    iotap = np.arange(128, dtype=np.float32).reshape(1, 128)

    in_maps = []
    for c in range(N_CORES):
        lo = c * SHARD
        slabE = np.zeros((SHPAD, D), np.float32)
        slabE[:SHARD] = -ET[lo:lo + SHARD]
        slabA = np.zeros((SHPAD, K), np.float32)
        slabA[:SHARD] = A[:, lo:lo + SHARD].T
        fused = np.concatenate([slabE, slabA], axis=1)        # (SHPAD, CW)
        get = np.ascontiguousarray(
            fused.reshape(NCH, 128, CW).transpose(1, 0, 2).reshape(128, FLATW))
        getbf = np.ascontiguousarray(
            slabE.reshape(NCH, 128, D).transpose(1, 0, 2).reshape(128, EW)
        ).astype(ml_dtypes.bfloat16)
        ash = np.zeros((K, SHPAD), np.float32)
        ash[:, :SHARD] = A[:, lo:lo + SHARD]
        in_maps.append({
            "get": get,
            "getbf": getbf,
            "a_hi": ash.astype(ml_dtypes.bfloat16),
            "identbf": identb,
            "identf": identf,
            "ptinit": ptinit,
            "jbase": np.array([[float(lo)]], np.float32),
            "padmask": padmask,
            "iotap": iotap,
        })
    return in_maps


def kernel(embedding, basis_init, activation_init, k, _want_debug=False):
    if "nc" not in _CACHE:
        _CACHE["nc"] = build_kernel()
    nc = _CACHE["nc"]
    in_maps = _prep_inputs(embedding, basis_init, activation_init)
    res = run_bass_kernel_spmd(nc, in_maps, core_ids=list(range(N_CORES)))
    r0 = res.results[0]
    out_b = np.ascontiguousarray(r0["out_bt"].T)      # (D, K)
    out_a = np.asarray(activation_init, dtype=np.float32)
    _CACHE["last_res"] = res
    _CACHE["last_dbg"] = r0["out_dbg"]
    if _want_debug:
        return (out_b, out_a), r0["out_dbg"]
    return out_b, out_a


# revision 7
# speedup vs baseline: 1.3160x; 1.0710x over previous
"""Trainium2 Bass kernel for nn_DiarizationModel: 10 Adam iterations of
L1-basis fitting. T=50000 sharded over 8 cores; basis replicated.

Per 128-row T-chunk the PE computes psum = (B@A - E)^T via a bf16
identity-copy matmul of -E^T plus a bf16 B matmul; the vector engine does a
fused abs-reduce to per-column L1 sums. Argmax via Max8/MaxIndex + PE
transpose; 8-core AllGather of (max, j) pairs; winner column fetched by
dynamic-offset DMA from a replicated fp32 table; gradient, Adam and shrink
run replicated on all cores in fp32.
"""
import math
import numpy as np
import ml_dtypes

import concourse.bass as bass
import concourse.bacc as bacc
import concourse.mybir as mybir
import concourse.tile as tile
from concourse.bass_utils import run_bass_kernel_spmd

F32 = mybir.dt.float32
BF16 = mybir.dt.bfloat16
U32 = mybir.dt.uint32
I32 = mybir.dt.int32
AX = mybir.AxisListType
ALU = mybir.AluOpType
ACT = mybir.ActivationFunctionType

N_CORES = 8
D = 512
K = 16
T = 50000
SHARD = T // N_CORES        # 6250
NCH = 49                    # chunks of 128 T-rows per core
SHPAD = NCH * 128           # 6272
CW = D + K                  # 528: fused winner row (negE | A)
EW = NCH * D                # 25088 bf16 matmul tensor width

LAMBDA1 = 0.3366
LR = 0.1
CSH = LR * LAMBDA1
B1, B2, EPS = 0.9, 0.999, 1e-8
N_ITERS = 10
NEG_BIG = -1.0e30
N_BA_TERMS = 1              # 1: A_hi*B_hi ; 2: + A_hi*B_lo


def build_kernel():
    nc = bacc.Bacc(trn_type="TRN2", num_devices=N_CORES)

    gf_d = nc.dram_tensor("getfull", [T, CW], F32, kind="ExternalInput")
    getbf_d = nc.dram_tensor("getbf", [128, EW], BF16, kind="ExternalInput")
    ahi_d = nc.dram_tensor("a_hi", [K, SHPAD], BF16, kind="ExternalInput")
    idb_d = nc.dram_tensor("identbf", [128, 128], BF16, kind="ExternalInput")
    idf_d = nc.dram_tensor("identf", [128, 128], F32, kind="ExternalInput")
    pt_d = nc.dram_tensor("ptinit", [K, D], F32, kind="ExternalInput")
    jb_d = nc.dram_tensor("jbase", [1, 1], F32, kind="ExternalInput")
    pm_d = nc.dram_tensor("padmask", [128, 1], F32, kind="ExternalInput")
    io_d = nc.dram_tensor("iotap", [1, 128], F32, kind="ExternalInput")

    obt_d = nc.dram_tensor("out_bt", [K, D], F32, kind="ExternalOutput")
    dbg_d = nc.dram_tensor("out_dbg", [1, 32], F32, kind="ExternalOutput")

    with tile.TileContext(nc) as tc:
        with tc.tile_pool(name="per", bufs=1) as per, \
             tc.tile_pool(name="scr", bufs=2) as scr, \
             tc.tile_pool(name="pmain", bufs=2, space="PSUM") as pmain, \
             tc.tile_pool(name="ptail", bufs=3, space="PSUM") as ptail, \
             tc.tile_pool(name="dram", bufs=2, space="DRAM") as dram:

            # ---- persistent tiles ----
            getbf = per.tile([128, EW], BF16)
            a_hi = per.tile([K, SHPAD], BF16)
            identb = per.tile([128, 128], BF16)
            identf = per.tile([128, 128], F32)
            padmask = per.tile([128, 1], F32)
            iotap = per.tile([1, 128], F32)
            jbase = per.tile([1, 1], F32)
            Ct = per.tile([128, 64], F32)
            PT = per.tile([K, D], F32)
            mT = per.tile([K, D], F32)      # m / (1-B1)
            vT = per.tile([K, D], F32)      # v / (1-B2)
            BT = per.tile([K, D], F32)
            BTh = per.tile([K, D], BF16)
            BTl = per.tile([K, D], BF16)
            smult = per.tile([K, D], F32)
            sgnP = per.tile([K, D], F32)
            mask_l1 = per.tile([K, 1], F32)
            dbg = per.tile([1, 32], F32)

            idf = identf[:]

            # ---- loads ----
            NSLAB = 7
            for s in range(NSLAB):
                w = EW // NSLAB
                nc.sync.dma_start(getbf[:, s * w:(s + 1) * w],
                                  getbf_d[:, s * w:(s + 1) * w])
            nc.sync.dma_start(a_hi[:], ahi_d[:])
            nc.sync.dma_start(identb[:], idb_d[:])
            nc.sync.dma_start(identf[:], idf_d[:])
            nc.sync.dma_start(padmask[:], pm_d[:])
            nc.sync.dma_start(iotap[:], io_d[:])
            nc.sync.dma_start(jbase[:], jb_d[:])
            nc.sync.dma_start(PT[:], pt_d[:])

            nc.vector.memset(mT[:], 0.0)
            nc.vector.memset(vT[:], 0.0)
            nc.vector.memset(Ct[:], NEG_BIG)
            nc.vector.memset(dbg[:], 0.0)
            nc.vector.tensor_copy(BT[:], PT[:])          # iter-1 B = P
            nc.scalar.copy(BTh[:], BT[:])
            if N_BA_TERMS >= 2:
                nc.vector.scalar_tensor_tensor(BTl[:], BTh[:], -1.0, BT[:],
                                               op0=ALU.mult, op1=ALU.add)
            nc.scalar.sign(sgnP[:], PT[:])

            # PE warm-up touches: absorb slab DMA waits one at a time
            for s in range(NSLAB):
                wm = ptail.tile([1, 128], BF16, name="wm", tag="tail")
                nc.tensor.transpose(wm[:], getbf[:, s * (EW // NSLAB):
                                                 s * (EW // NSLAB) + 1],
                                    identb[:])
            gsem = nc.alloc_semaphore("gsem")
            gcnt = 0

            for t in range(1, N_ITERS + 1):
                c2t = 1.0 - B2 ** t
                kt = LR * (1.0 - B1) / (1.0 - B1 ** t)
                sqscale = (1.0 - B2) / c2t

                # ---------- g2 prep: k* = argmax colsum|B| (only needs BT) ----
                cb = scr.tile([K, 1], F32, name="cb", tag="cb")
                nc.vector.tensor_reduce(cb[:], BT[:], axis=AX.X,
                                        op=ALU.add, apply_absolute_value=True)
                cbT = ptail.tile([1, K], F32, name="cbT", tag="tail")
                nc.tensor.transpose(cbT[:], cb[:], idf[0:K, 0:K])
                rcb = scr.tile([1, K], F32, name="rcb", tag="rcb")
                nc.scalar.copy(rcb[:], cbT[:])
                cb8 = scr.tile([1, 8], F32, name="cb8", tag="cb8")
                nc.vector.max(cb8[:], rcb[:])
                kmr = scr.tile([1, K], F32, name="kmr", tag="kmr")
                nc.vector.tensor_scalar(kmr[:], rcb[:], cb8[:, 0:1], None,
                                        op0=ALU.is_ge)
                kmT = ptail.tile([K, 1], F32, name="kmT", tag="tail")
                nc.tensor.transpose(kmT[:], kmr[:], idf[0:1, 0:1])
                nc.scalar.mul(mask_l1[:], kmT[:], LAMBDA1)

                # ---------- main pass: colsums of |B A - E| ----------
                for grp in range(25):
                    ng = 2 if grp < 24 else 1
                    ps = pmain.tile([128, 1024], F32, name="ps", tag="mainps")
                    for q in range(ng):
                        ch = grp * 2 + q
                        o = ps[:, q * 512:(q + 1) * 512]
                        nc.tensor.matmul(o, identb[:],
                                         getbf[:, ch * D:(ch + 1) * D],
                                         start=True, stop=False)
                        nc.tensor.matmul(o, a_hi[:, ch * 128:(ch + 1) * 128],
                                         BTh[:], start=False,
                                         stop=(N_BA_TERMS == 1))
                        if N_BA_TERMS >= 2:
                            nc.tensor.matmul(
                                o, a_hi[:, ch * 128:(ch + 1) * 128],
                                BTl[:], start=False, stop=True)
                    view = ps[:, 0:ng * 512].rearrange("p (n d) -> p n d", d=512)
                    nc.vector.tensor_reduce(Ct[:, grp * 2: grp * 2 + ng], view,
                                            axis=AX.X, op=ALU.add,
                                            apply_absolute_value=True)

                # mask pad rows of last chunk
                nc.vector.tensor_scalar(Ct[:, 48:49], Ct[:, 48:49],
                                        padmask[:], None, op0=ALU.add)

                # ---------- local argmax ----------
                m8 = scr.tile([128, 8], F32, name="m8", tag="m8")
                i8 = scr.tile([128, 8], U32, name="i8", tag="i8")
                nc.vector.max(m8[:], Ct[:])
                nc.vector.max_index(i8[:], m8[:], Ct[:])
                ixf = scr.tile([128, 1], F32, name="ixf", tag="ixf")
                nc.vector.tensor_copy(ixf[:], i8[:, 0:1])
                tpm = ptail.tile([1, 128], F32, name="tpm", tag="tail")
                tpi = ptail.tile([1, 128], F32, name="tpi", tag="tail")
                nc.tensor.transpose(tpm[:], m8[:, 0:1], idf)
                nc.tensor.transpose(tpi[:], ixf[:], idf)
                rmx = scr.tile([1, 128], F32, name="rmx", tag="rmx")
                rix = scr.tile([1, 128], F32, name="rix", tag="rix")
                nc.scalar.copy(rmx[:], tpm[:])
                nc.scalar.copy(rix[:], tpi[:])
                g8 = scr.tile([1, 8], F32, name="g8", tag="g8")
                gi8 = scr.tile([1, 8], U32, name="gi8", tag="gi8")
                nc.vector.max(g8[:], rmx[:])
                nc.vector.max_index(gi8[:], g8[:], rmx[:])
                ploc = scr.tile([1, 1], F32, name="ploc", tag="ploc")
                nc.vector.tensor_copy(ploc[:], gi8[:, 0:1])
                # cloc = rix[ploc]
                eqm = scr.tile([1, 128], F32, name="eqm", tag="eqm")
                nc.vector.tensor_scalar(eqm[:], iotap[:], ploc[:], None,
                                        op0=ALU.is_equal)
                nc.vector.tensor_tensor(out=eqm[:], in0=eqm[:], in1=rix[:],
                                        op=ALU.mult)
                cloc = scr.tile([1, 1], F32, name="cloc", tag="cloc")
                nc.vector.tensor_reduce(cloc[:], eqm[:], axis=AX.X, op=ALU.add)
                # jglob = jbase + cloc*128 + ploc
                jg = scr.tile([1, 1], F32, name="jg", tag="jg")
                nc.vector.scalar_tensor_tensor(jg[:], cloc[:], 128.0, ploc[:],
                                               op0=ALU.mult, op1=ALU.add)
                nc.vector.tensor_tensor(out=jg[:], in0=jg[:], in1=jbase[:],
                                        op=ALU.add)

                # ---------- AllGather of (maxval, jglob) ----------
                blob = scr.tile([1, 8], F32, name="blob", tag="blob")
                nc.scalar.copy(blob[:, 0:1], g8[:, 0:1])
                nc.scalar.copy(blob[:, 1:2], jg[:])
                agi = dram.tile([1, 8], F32, name="agi", tag="agi")
                ago = dram.tile([8, 8], F32, name="ago", tag="ago",
                                addr_space="Shared")
                nc.sync.dma_start(agi[:], blob[:])
                nc.gpsimd.collective_compute(
                    "AllGather", ALU.bypass,
                    replica_groups=[list(range(N_CORES))],
                    ins=[agi[:]], outs=[ago[:]])

                # ---------- winner ----------
                vals8 = scr.tile([1, 8], F32, name="vals8", tag="vals8")
                jg8 = scr.tile([1, 8], F32, name="jg8", tag="jg8")
                with nc.allow_non_contiguous_dma(reason="8-elem gathers"):
                    nc.sync.dma_start(vals8[:], ago[:, 0:1].transpose([1, 0]))
                    nc.sync.dma_start(jg8[:], ago[:, 1:2].transpose([1, 0]))
                w8 = scr.tile([1, 8], F32, name="w8", tag="w8")
                wi8 = scr.tile([1, 8], U32, name="wi8", tag="wi8")
                nc.vector.max(w8[:], vals8[:])
                nc.vector.max_index(wi8[:], w8[:], vals8[:])
                wf = scr.tile([1, 1], F32, name="wf", tag="wf")
                nc.vector.tensor_copy(wf[:], wi8[:, 0:1])
                eqw = scr.tile([1, 8], F32, name="eqw", tag="eqw")
                nc.vector.tensor_scalar(eqw[:], iotap[:, 0:8], wf[:], None,
                                        op0=ALU.is_equal)
                nc.vector.tensor_tensor(out=eqw[:], in0=eqw[:], in1=jg8[:],
                                        op=ALU.mult)
                jwin = scr.tile([1, 1], F32, name="jwin", tag="jwin")
                nc.vector.tensor_reduce(jwin[:], eqw[:], axis=AX.X, op=ALU.add)
                jwi = scr.tile([1, 1], I32, name="jwi", tag="jwi")
                nc.vector.tensor_copy(jwi[:], jwin[:])
                winner = scr.tile([1, CW], F32, name="winner", tag="winner")
                with tc.tile_critical():
                    jv = nc.gpsimd.value_load(jwi[:])
                    gcnt += 16
                    nc.gpsimd.dma_start(
                        winner[:], gf_d[bass.ds(jv, 1), :]).then_inc(gsem, 16)
                    nc.gpsimd.wait_ge(gsem, gcnt)

                # debug capture
                nc.scalar.copy(dbg[:, t - 1:t], jwin[:])
                nc.scalar.copy(dbg[:, 9 + t:10 + t], w8[:, 0:1])

                # ---------- gradient (fp32 exact path) ----------
                acT = ptail.tile([K, 1], F32, name="acT", tag="tail")
                nc.tensor.transpose(acT[:], winner[:, D:D + K], idf[0:1, 0:1])
                acol = scr.tile([K, 1], F32, name="acol", tag="acol")
                nc.scalar.copy(acol[:], acT[:])
                ba = ptail.tile([1, D], F32, name="ba", tag="tail")
                nc.tensor.matmul(ba[:], acol[:], BT[:], start=True, stop=True)
                u = scr.tile([1, D], F32, name="u", tag="u")
                nc.vector.tensor_tensor(out=u[:], in0=winner[:, 0:D],
                                        in1=ba[:], op=ALU.add)
                srow = scr.tile([1, D], F32, name="srow", tag="srow")
                nc.scalar.sign(srow[:], u[:])
                g1T = ptail.tile([K, D], F32, name="g1T", tag="tail")
                nc.tensor.matmul(g1T[:], winner[:, D:D + K], srow[:],
                                 start=True, stop=True)

                gT = scr.tile([K, D], F32, name="gT", tag="gT")
                nc.vector.scalar_tensor_tensor(gT[:], sgnP[:], mask_l1[:],
                                               g1T[:], op0=ALU.mult, op1=ALU.add)
                if t >= 2:
                    nc.vector.tensor_tensor(out=gT[:], in0=gT[:], in1=smult[:],
                                            op=ALU.mult)

                # ---------- Adam (rescaled moments; eps folded away) --------
                sq = scr.tile([K, D], F32, name="sq", tag="sq")
                nc.scalar.square(sq[:], gT[:])
                nc.vector.scalar_tensor_tensor(vT[:], vT[:], B2, sq[:],
                                               op0=ALU.mult, op1=ALU.add)
                nc.vector.scalar_tensor_tensor(mT[:], mT[:], B1, gT[:],
                                               op0=ALU.mult, op1=ALU.add)
                dn = scr.tile([K, D], F32, name="dn", tag="dn")
                nc.scalar.activation(dn[:], vT[:], ACT.Sqrt, scale=sqscale)
                rcp = scr.tile([K, D], F32, name="rcp", tag="rcp")
                rscr = scr.tile([K, D], F32, name="rscr", tag="rscr")
                nc.vector.reciprocal_approx_accurate(rcp[:], dn[:], rscr[:])
                nc.vector.tensor_tensor(out=rcp[:], in0=mT[:], in1=rcp[:],
                                        op=ALU.mult)
                nc.vector.scalar_tensor_tensor(PT[:], rcp[:], -kt, PT[:],
                                               op0=ALU.mult, op1=ALU.add)

                # ---------- shrink / next-iteration B ----------
                t1 = scr.tile([K, D], F32, name="t1", tag="t1")
                nc.vector.tensor_scalar_sub(t1[:], PT[:], CSH)
                nc.scalar.sign(sgnP[:], PT[:])
                ab1 = scr.tile([K, D], F32, name="ab1", tag="ab1")
                nc.scalar.activation(ab1[:], t1[:], ACT.Abs)
                if t < N_ITERS:
                    nc.vector.tensor_tensor(out=BTh[:], in0=sgnP[:], in1=ab1[:],
                                            op=ALU.mult)
                    nc.vector.tensor_tensor(out=BT[:], in0=sgnP[:], in1=ab1[:],
                                            op=ALU.mult)
                    if N_BA_TERMS >= 2:
                        nc.vector.scalar_tensor_tensor(
                            BTl[:], BTh[:], -1.0, BT[:],
                            op0=ALU.mult, op1=ALU.add)
                    sg1 = scr.tile([K, D], F32, name="sg1", tag="sg1")
                    nc.scalar.sign(sg1[:], t1[:])
                    nc.vector.tensor_tensor(out=smult[:], in0=sgnP[:],
                                            in1=sg1[:], op=ALU.mult)
                else:
                    nc.vector.tensor_tensor(out=BT[:], in0=sgnP[:], in1=ab1[:],
                                            op=ALU.mult)
                    nc.sync.dma_start(obt_d[:], BT[:])
                    nc.sync.dma_start(dbg_d[:], dbg[:])

    nc.compile()
    return nc


_CACHE = {}


def _prep_inputs(embedding, basis_init, activation_init):
    E = np.ascontiguousarray(embedding, dtype=np.float32)
    A = np.ascontiguousarray(activation_init, dtype=np.float32)
    B0 = np.ascontiguousarray(basis_init, dtype=np.float32)
    ET = np.ascontiguousarray(E.T)              # (T, D)
    identf = np.eye(128, dtype=np.float32)
    identb = identf.astype(ml_dtypes.bfloat16)
    ptinit = np.ascontiguousarray(B0.T)         # (K, D)
    padmask = np.zeros((128, 1), np.float32)
    padmask[SHARD - 48 * 128:, :] = NEG_BIG
    iotap = np.arange(128, dtype=np.float32).reshape(1, 128)

    getfull = np.ascontiguousarray(
        np.concatenate([-ET, A.T], axis=1))     # (T, CW) shared by all cores

    in_maps = []
    for c in range(N_CORES):
        lo = c * SHARD
        slabE = np.zeros((SHPAD, D), np.float32)
        slabE[:SHARD] = -ET[lo:lo + SHARD]
        getbf = np.ascontiguousarray(
            slabE.reshape(NCH, 128, D).transpose(1, 0, 2).reshape(128, EW)
        ).astype(ml_dtypes.bfloat16)
        ash = np.zeros((K, SHPAD), np.float32)
        ash[:, :SHARD] = A[:, lo:lo + SHARD]
        in_maps.append({
            "getfull": getfull,
            "getbf": getbf,
            "a_hi": ash.astype(ml_dtypes.bfloat16),
            "identbf": identb,
            "identf": identf,
            "ptinit": ptinit,
            "jbase": np.array([[float(lo)]], np.float32),
            "padmask": padmask,
            "iotap": iotap,
        })
    return in_maps


def kernel(embedding, basis_init, activation_init, k, _want_debug=False):
    if "nc" not in _CACHE:
        _CACHE["nc"] = build_kernel()
    nc = _CACHE["nc"]
    in_maps = _prep_inputs(embedding, basis_init, activation_init)
    res = run_bass_kernel_spmd(nc, in_maps, core_ids=list(range(N_CORES)))
    r0 = res.results[0]
    out_b = np.ascontiguousarray(r0["out_bt"].T)      # (D, K)
    out_a = np.asarray(activation_init, dtype=np.float32)
    _CACHE["last_res"] = res
    _CACHE["last_dbg"] = r0["out_dbg"]
    if _want_debug:
        return (out_b, out_a), r0["out_dbg"]
    return out_b, out_a


# revision 9
# speedup vs baseline: 1.4032x; 1.0662x over previous
"""Trainium2 Bass kernel for nn_DiarizationModel: 10 Adam iterations of
L1-basis fitting. T=50000 sharded over 8 cores; basis replicated.

Per 128-row T-chunk the PE computes psum = (B@A - E)^T via a bf16
identity-copy matmul of -E^T plus a bf16 B matmul; the vector engine does a
fused abs-reduce to per-column L1 sums. Argmax via Max8/MaxIndex + PE
transpose; 8-core AllGather of (max, j) pairs; winner column fetched by
dynamic-offset DMA from a replicated fp32 table; gradient, Adam and shrink
run replicated on all cores in fp32.
"""
import math
import numpy as np
import ml_dtypes

import concourse.bass as bass
import concourse.bacc as bacc
import concourse.mybir as mybir
import concourse.tile as tile
from concourse.bass_utils import run_bass_kernel_spmd

F32 = mybir.dt.float32
BF16 = mybir.dt.bfloat16
U32 = mybir.dt.uint32
I32 = mybir.dt.int32
AX = mybir.AxisListType
ALU = mybir.AluOpType
ACT = mybir.ActivationFunctionType

N_CORES = 8
D = 512
K = 16
T = 50000
SHARD = T // N_CORES        # 6250
NCH = 49                    # chunks of 128 T-rows per core
SHPAD = NCH * 128           # 6272
CW = D + K                  # 528: fused winner row (negE | A)
EW = NCH * D                # 25088 bf16 matmul tensor width

LAMBDA1 = 0.3366
LR = 0.1
CSH = LR * LAMBDA1
B1, B2, EPS = 0.9, 0.999, 1e-8
N_ITERS = 10
NEG_BIG = -1.0e30
N_BA_TERMS = 1              # 1: A_hi*B_hi ; 2: + A_hi*B_lo


def build_kernel():
    nc = bacc.Bacc(trn_type="TRN2", num_devices=N_CORES)

    gf_d = nc.dram_tensor("getfull", [T, CW], F32, kind="ExternalInput")
    getbf_d = nc.dram_tensor("getbf", [128, EW], BF16, kind="ExternalInput")
    ahi_d = nc.dram_tensor("a_hi", [128, SHPAD], BF16, kind="ExternalInput")
    rep_d = nc.dram_tensor("repmat", [K, 128], BF16, kind="ExternalInput")
    idb_d = nc.dram_tensor("identbf", [128, 128], BF16, kind="ExternalInput")
    idf_d = nc.dram_tensor("identf", [128, 128], F32, kind="ExternalInput")
    pt_d = nc.dram_tensor("ptinit", [K, D], F32, kind="ExternalInput")
    jb_d = nc.dram_tensor("jbase", [1, 1], F32, kind="ExternalInput")
    pm_d = nc.dram_tensor("padmask", [128, 1], F32, kind="ExternalInput")
    io_d = nc.dram_tensor("iotap", [1, 128], F32, kind="ExternalInput")

    obt_d = nc.dram_tensor("out_bt", [K, D], F32, kind="ExternalOutput")
    dbg_d = nc.dram_tensor("out_dbg", [1, 32], F32, kind="ExternalOutput")

    with tile.TileContext(nc) as tc:
        with tc.tile_pool(name="per", bufs=1) as per, \
             tc.tile_pool(name="scr", bufs=2) as scr, \
             tc.tile_pool(name="pmain", bufs=2, space="PSUM") as pmain, \
             tc.tile_pool(name="ptail", bufs=2, space="PSUM") as ptail, \
             tc.tile_pool(name="dram", bufs=2, space="DRAM") as dram:

            # ---- persistent tiles ----
            getbf = per.tile([128, EW], BF16)
            a_hi = per.tile([128, SHPAD], BF16)
            repm = per.tile([K, 128], BF16)
            BTh4 = per.tile([128, D], BF16)
            identb = per.tile([128, 128], BF16)
            identf = per.tile([128, 128], F32)
            padmask = per.tile([128, 1], F32)
            iotap = per.tile([1, 128], F32)
            jbase = per.tile([1, 1], F32)
            Ct = per.tile([128, 64], F32)
            PT = per.tile([K, D], F32)
            mT = per.tile([K, D], F32)      # m / (1-B1)
            vT = per.tile([K, D], F32)      # v / (1-B2)
            BT = per.tile([K, D], F32)
            BTh = per.tile([K, D], BF16)
            BTl = per.tile([K, D], BF16)
            smult = per.tile([K, D], F32)
            sgnP = per.tile([K, D], F32)
            mask_l1 = per.tile([K, 1], F32)
            dbg = per.tile([1, 32], F32)

            idf = identf[:]

            # ---- loads ----
            NSLAB = 7
            for s in range(NSLAB):
                w = EW // NSLAB
                nc.sync.dma_start(getbf[:, s * w:(s + 1) * w],
                                  getbf_d[:, s * w:(s + 1) * w])
            nc.sync.dma_start(a_hi[:], ahi_d[:])
            nc.sync.dma_start(repm[:], rep_d[:])
            nc.sync.dma_start(identb[:], idb_d[:])
            nc.sync.dma_start(identf[:], idf_d[:])
            nc.sync.dma_start(padmask[:], pm_d[:])
            nc.sync.dma_start(iotap[:], io_d[:])
            nc.sync.dma_start(jbase[:], jb_d[:])
            nc.sync.dma_start(PT[:], pt_d[:])

            nc.vector.memset(mT[:], 0.0)
            nc.vector.memset(vT[:], 0.0)
            nc.vector.memset(Ct[:], NEG_BIG)
            nc.vector.memset(dbg[:], 0.0)
            nc.vector.tensor_copy(BT[:], PT[:])          # iter-1 B = P
            nc.scalar.copy(BTh[:], BT[:])
            rep_ps0 = ptail.tile([128, D], F32, name="rep_ps", tag="tail")
            nc.tensor.matmul(rep_ps0[:], repm[:], BTh[:], start=True, stop=True)
            nc.scalar.copy(BTh4[:], rep_ps0[:])
            if N_BA_TERMS >= 2:
                nc.vector.scalar_tensor_tensor(BTl[:], BTh[:], -1.0, BT[:],
                                               op0=ALU.mult, op1=ALU.add)
            nc.scalar.sign(sgnP[:], PT[:])

            # PE warm-up touches: absorb slab DMA waits one at a time
            for s in range(NSLAB):
                wm = ptail.tile([1, 128], BF16, name="wm", tag="tail")
                nc.tensor.transpose(wm[:], getbf[:, s * (EW // NSLAB):
                                                 s * (EW // NSLAB) + 1],
                                    identb[:])
            gsem = nc.alloc_semaphore("gsem")
            gcnt = 0

            for t in range(1, N_ITERS + 1):
                c2t = 1.0 - B2 ** t
                kt = LR * (1.0 - B1) / (1.0 - B1 ** t)
                sqscale = (1.0 - B2) / c2t

                # ---------- g2 prep: k* = argmax colsum|B| (only needs BT) ----
                cb = scr.tile([K, 1], F32, name="cb", tag="cb")
                nc.vector.tensor_reduce(cb[:], BT[:], axis=AX.X,
                                        op=ALU.add, apply_absolute_value=True)
                cbT = ptail.tile([1, K], F32, name="cbT", tag="tail")
                nc.tensor.transpose(cbT[:], cb[:], idf[0:K, 0:K])
                rcb = scr.tile([1, K], F32, name="rcb", tag="rcb")
                nc.scalar.copy(rcb[:], cbT[:])
                cb8 = scr.tile([1, 8], F32, name="cb8", tag="cb8")
                nc.vector.max(cb8[:], rcb[:])
                kmr = scr.tile([1, K], F32, name="kmr", tag="kmr")
                nc.vector.tensor_scalar(kmr[:], rcb[:], cb8[:, 0:1], None,
                                        op0=ALU.is_ge)
                kmT = ptail.tile([K, 1], F32, name="kmT", tag="tail")
                nc.tensor.transpose(kmT[:], kmr[:], idf[0:1, 0:1])
                nc.scalar.mul(mask_l1[:], kmT[:], LAMBDA1)

                # ---------- main pass: colsums of |B A - E| ----------
                # packs of 3 chunks; BA matmuls run concurrently in PE
                # row-groups 0/32/64 via tile_position; identity-copy matmuls
                # lay -E^T into PSUM first. Reduce split: vector fused
                # abs-reduce for most packs, scalar Abs+accum for the rest.
                NPACK = 17
                N_ACT_PACKS = 6          # packs whose reduce runs on scalar
                for pk in range(NPACK):
                    ng = 3 if pk < NPACK - 1 else 1
                    ps = pmain.tile([128, 1536], F32, name="ps", tag="mainps")
                    for q in range(ng):
                        ch = pk * 3 + q
                        o = ps[:, q * 512:(q + 1) * 512]
                        nc.tensor.matmul(o, identb[:],
                                         getbf[:, ch * D:(ch + 1) * D],
                                         start=True, stop=False)
                    for q in range(ng):
                        ch = pk * 3 + q
                        o = ps[:, q * 512:(q + 1) * 512]
                        nc.tensor.matmul(
                            o, a_hi[32 * q:32 * q + K, ch * 128:(ch + 1) * 128],
                            BTh4[32 * q:32 * q + K, :], start=False, stop=True,
                            tile_position=(32 * q, 0))
                    ch0 = pk * 3
                    if NPACK - 1 - N_ACT_PACKS <= pk < NPACK - 1:
                        for q in range(ng):
                            o = ps[:, q * 512:(q + 1) * 512]
                            nc.scalar.activation(o, o, ACT.Abs,
                                                 accum_out=Ct[:, ch0 + q:ch0 + q + 1])
                    else:
                        view = ps[:, 0:ng * 512].rearrange("p (n d) -> p n d",
                                                           d=512)
                        nc.vector.tensor_reduce(Ct[:, ch0: ch0 + ng], view,
                                                axis=AX.X, op=ALU.add,
                                                apply_absolute_value=True)

                # mask pad rows of last chunk
                nc.vector.tensor_scalar(Ct[:, 48:49], Ct[:, 48:49],
                                        padmask[:], None, op0=ALU.add)

                # ---------- local argmax ----------
                m8 = scr.tile([128, 8], F32, name="m8", tag="m8")
                i8 = scr.tile([128, 8], U32, name="i8", tag="i8")
                nc.vector.max(m8[:], Ct[:])
                nc.vector.max_index(i8[:], m8[:], Ct[:])
                ixf = scr.tile([128, 1], F32, name="ixf", tag="ixf")
                nc.vector.tensor_copy(ixf[:], i8[:, 0:1])
                tpm = ptail.tile([1, 128], F32, name="tpm", tag="tail")
                tpi = ptail.tile([1, 128], F32, name="tpi", tag="tail")
                nc.tensor.transpose(tpm[:], m8[:, 0:1], idf)
                nc.tensor.transpose(tpi[:], ixf[:], idf)
                rmx = scr.tile([1, 128], F32, name="rmx", tag="rmx")
                rix = scr.tile([1, 128], F32, name="rix", tag="rix")
                nc.scalar.copy(rmx[:], tpm[:])
                nc.scalar.copy(rix[:], tpi[:])
                g8 = scr.tile([1, 8], F32, name="g8", tag="g8")
                gi8 = scr.tile([1, 8], U32, name="gi8", tag="gi8")
                nc.vector.max(g8[:], rmx[:])
                nc.vector.max_index(gi8[:], g8[:], rmx[:])
                ploc = scr.tile([1, 1], F32, name="ploc", tag="ploc")
                nc.vector.tensor_copy(ploc[:], gi8[:, 0:1])
                # cloc = rix[ploc]
                eqm = scr.tile([1, 128], F32, name="eqm", tag="eqm")
                nc.vector.tensor_scalar(eqm[:], iotap[:], ploc[:], None,
                                        op0=ALU.is_equal)
                nc.vector.tensor_tensor(out=eqm[:], in0=eqm[:], in1=rix[:],
                                        op=ALU.mult)
                cloc = scr.tile([1, 1], F32, name="cloc", tag="cloc")
                nc.vector.tensor_reduce(cloc[:], eqm[:], axis=AX.X, op=ALU.add)
                # jglob = jbase + cloc*128 + ploc
                jg = scr.tile([1, 1], F32, name="jg", tag="jg")
                nc.vector.scalar_tensor_tensor(jg[:], cloc[:], 128.0, ploc[:],
                                               op0=ALU.mult, op1=ALU.add)
                nc.vector.tensor_tensor(out=jg[:], in0=jg[:], in1=jbase[:],
                                        op=ALU.add)

                # ---------- AllGather of (maxval, jglob) ----------
                blob = scr.tile([1, 8], F32, name="blob", tag="blob")
                nc.scalar.copy(blob[:, 0:1], g8[:, 0:1])
                nc.scalar.copy(blob[:, 1:2], jg[:])
                agi = dram.tile([1, 8], F32, name="agi", tag="agi")
                ago = dram.tile([8, 8], F32, name="ago", tag="ago",
                                addr_space="Shared")
                nc.sync.dma_start(agi[:], blob[:])
                nc.gpsimd.collective_compute(
                    "AllGather", ALU.bypass,
                    replica_groups=[list(range(N_CORES))],
                    ins=[agi[:]], outs=[ago[:]])

                # ---------- winner ----------
                vals8 = scr.tile([1, 8], F32, name="vals8", tag="vals8")
                jg8 = scr.tile([1, 8], F32, name="jg8", tag="jg8")
                with nc.allow_non_contiguous_dma(reason="8-elem gathers"):
                    nc.sync.dma_start(vals8[:], ago[:, 0:1].transpose([1, 0]))
                    nc.sync.dma_start(jg8[:], ago[:, 1:2].transpose([1, 0]))
                w8 = scr.tile([1, 8], F32, name="w8", tag="w8")
                wi8 = scr.tile([1, 8], U32, name="wi8", tag="wi8")
                nc.vector.max(w8[:], vals8[:])
                nc.vector.max_index(wi8[:], w8[:], vals8[:])
                wf = scr.tile([1, 1], F32, name="wf", tag="wf")
                nc.vector.tensor_copy(wf[:], wi8[:, 0:1])
                eqw = scr.tile([1, 8], F32, name="eqw", tag="eqw")
                nc.vector.tensor_scalar(eqw[:], iotap[:, 0:8], wf[:], None,
                                        op0=ALU.is_equal)
                nc.vector.tensor_tensor(out=eqw[:], in0=eqw[:], in1=jg8[:],
                                        op=ALU.mult)
                jwin = scr.tile([1, 1], F32, name="jwin", tag="jwin")
                nc.vector.tensor_reduce(jwin[:], eqw[:], axis=AX.X, op=ALU.add)
                jwi = scr.tile([1, 1], I32, name="jwi", tag="jwi")
                nc.vector.tensor_copy(jwi[:], jwin[:])
                winner = scr.tile([1, CW], F32, name="winner", tag="winner")
                with tc.tile_critical():
                    jv = nc.gpsimd.value_load(jwi[:])
                    gcnt += 16
                    nc.gpsimd.dma_start(
                        winner[:], gf_d[bass.ds(jv, 1), :]).then_inc(gsem, 16)
                    nc.gpsimd.wait_ge(gsem, gcnt)

                # debug capture
                nc.scalar.copy(dbg[:, t - 1:t], jwin[:])
                nc.scalar.copy(dbg[:, 9 + t:10 + t], w8[:, 0:1])

                # ---------- gradient (fp32 exact path) ----------
                acT = ptail.tile([K, 1], F32, name="acT", tag="tail")
                nc.tensor.transpose(acT[:], winner[:, D:D + K], idf[0:1, 0:1])
                acol = scr.tile([K, 1], F32, name="acol", tag="acol")
                nc.scalar.copy(acol[:], acT[:])
                ba = ptail.tile([1, D], F32, name="ba", tag="tail")
                nc.tensor.matmul(ba[:], acol[:], BT[:], start=True, stop=True)
                u = scr.tile([1, D], F32, name="u", tag="u")
                nc.vector.tensor_tensor(out=u[:], in0=winner[:, 0:D],
                                        in1=ba[:], op=ALU.add)
                srow = scr.tile([1, D], F32, name="srow", tag="srow")
                nc.scalar.sign(srow[:], u[:])
                g1T = ptail.tile([K, D], F32, name="g1T", tag="tail")
                nc.tensor.matmul(g1T[:], winner[:, D:D + K], srow[:],
                                 start=True, stop=True)

                gT = scr.tile([K, D], F32, name="gT", tag="gT")
                nc.vector.scalar_tensor_tensor(gT[:], sgnP[:], mask_l1[:],
                                               g1T[:], op0=ALU.mult, op1=ALU.add)
                if t >= 2:
                    nc.vector.tensor_tensor(out=gT[:], in0=gT[:], in1=smult[:],
                                            op=ALU.mult)

                # ---------- Adam (rescaled moments; eps folded away) --------
                sq = scr.tile([K, D], F32, name="sq", tag="sq")
                nc.scalar.square(sq[:], gT[:])
                nc.vector.scalar_tensor_tensor(vT[:], vT[:], B2, sq[:],
                                               op0=ALU.mult, op1=ALU.add)
                nc.vector.scalar_tensor_tensor(mT[:], mT[:], B1, gT[:],
                                               op0=ALU.mult, op1=ALU.add)
                dn = scr.tile([K, D], F32, name="dn", tag="dn")
                nc.scalar.activation(dn[:], vT[:], ACT.Sqrt, scale=sqscale)
                rcp = scr.tile([K, D], F32, name="rcp", tag="rcp")
                rscr = scr.tile([K, D], F32, name="rscr", tag="rscr")
                nc.vector.reciprocal_approx_accurate(rcp[:], dn[:], rscr[:])
                nc.vector.tensor_tensor(out=rcp[:], in0=mT[:], in1=rcp[:],
                                        op=ALU.mult)
                nc.vector.scalar_tensor_tensor(PT[:], rcp[:], -kt, PT[:],
                                               op0=ALU.mult, op1=ALU.add)

                # ---------- shrink / next-iteration B ----------
                t1 = scr.tile([K, D], F32, name="t1", tag="t1")
                nc.vector.tensor_scalar_sub(t1[:], PT[:], CSH)
                nc.scalar.sign(sgnP[:], PT[:])
                ab1 = scr.tile([K, D], F32, name="ab1", tag="ab1")
                nc.scalar.activation(ab1[:], t1[:], ACT.Abs)
                if t < N_ITERS:
                    nc.vector.tensor_tensor(out=BTh[:], in0=sgnP[:], in1=ab1[:],
                                            op=ALU.mult)
                    rep_ps = ptail.tile([128, D], F32, name="rep_ps", tag="tail")
                    nc.tensor.matmul(rep_ps[:], repm[:], BTh[:],
                                     start=True, stop=True)
                    nc.scalar.copy(BTh4[:], rep_ps[:])
                    nc.vector.tensor_tensor(out=BT[:], in0=sgnP[:], in1=ab1[:],
                                            op=ALU.mult)
                    if N_BA_TERMS >= 2:
                        nc.vector.scalar_tensor_tensor(
                            BTl[:], BTh[:], -1.0, BT[:],
                            op0=ALU.mult, op1=ALU.add)
                    sg1 = scr.tile([K, D], F32, name="sg1", tag="sg1")
                    nc.scalar.sign(sg1[:], t1[:])
                    nc.vector.tensor_tensor(out=smult[:], in0=sgnP[:],
                                            in1=sg1[:], op=ALU.mult)
                else:
                    nc.vector.tensor_tensor(out=BT[:], in0=sgnP[:], in1=ab1[:],
                                            op=ALU.mult)
                    nc.sync.dma_start(obt_d[:], BT[:])
                    nc.sync.dma_start(dbg_d[:], dbg[:])

    nc.compile()
    return nc


_CACHE = {}


def _prep_inputs(embedding, basis_init, activation_init):
    E = np.ascontiguousarray(embedding, dtype=np.float32)
    A = np.ascontiguousarray(activation_init, dtype=np.float32)
    B0 = np.ascontiguousarray(basis_init, dtype=np.float32)
    ET = np.ascontiguousarray(E.T)              # (T, D)
    identf = np.eye(128, dtype=np.float32)
    identb = identf.astype(ml_dtypes.bfloat16)
    ptinit = np.ascontiguousarray(B0.T)         # (K, D)
    padmask = np.zeros((128, 1), np.float32)
    padmask[SHARD - 48 * 128:, :] = NEG_BIG
    iotap = np.arange(128, dtype=np.float32).reshape(1, 128)

    getfull = np.ascontiguousarray(
        np.concatenate([-ET, A.T], axis=1))     # (T, CW) shared by all cores
    repmat = np.zeros((K, 128), np.float32)
    for qq in range(4):
        repmat[np.arange(K), 32 * qq + np.arange(K)] = 1.0
    repmat = repmat.astype(ml_dtypes.bfloat16)

    in_maps = []
    for c in range(N_CORES):
        lo = c * SHARD
        slabE = np.zeros((SHPAD, D), np.float32)
        slabE[:SHARD] = -ET[lo:lo + SHARD]
        getbf = np.ascontiguousarray(
            slabE.reshape(NCH, 128, D).transpose(1, 0, 2).reshape(128, EW)
        ).astype(ml_dtypes.bfloat16)
        ash = np.zeros((128, SHPAD), np.float32)
        for qq in range(3):
            ash[32 * qq:32 * qq + K, :SHARD] = A[:, lo:lo + SHARD]
        in_maps.append({
            "getfull": getfull,
            "getbf": getbf,
            "a_hi": ash.astype(ml_dtypes.bfloat16),
            "repmat": repmat,
            "identbf": identb,
            "identf": identf,
            "ptinit": ptinit,
            "jbase": np.array([[float(lo)]], np.float32),
            "padmask": padmask,
            "iotap": iotap,
        })
    return in_maps


def kernel(embedding, basis_init, activation_init, k, _want_debug=False):
    if "nc" not in _CACHE:
        _CACHE["nc"] = build_kernel()
    nc = _CACHE["nc"]
    in_maps = _prep_inputs(embedding, basis_init, activation_init)
    res = run_bass_kernel_spmd(nc, in_maps, core_ids=list(range(N_CORES)))
    r0 = res.results[0]
    out_b = np.ascontiguousarray(r0["out_bt"].T)      # (D, K)
    out_a = np.asarray(activation_init, dtype=np.float32)
    _CACHE["last_res"] = res
    _CACHE["last_dbg"] = r0["out_dbg"]
    if _want_debug:
        return (out_b, out_a), r0["out_dbg"]
    return out_b, out_a


# revision 10
# speedup vs baseline: 1.5784x; 1.1249x over previous
"""Trainium2 Bass kernel for nn_DiarizationModel: 10 Adam iterations of
L1-basis fitting. T=50000 sharded over 8 cores; basis replicated.

Per 128-row T-chunk the PE computes psum = (B@A - E)^T via a bf16
identity-copy matmul of -E^T plus a bf16 B matmul; the vector engine does a
fused abs-reduce to per-column L1 sums. Argmax via Max8/MaxIndex + PE
transpose; 8-core AllGather of (max, j) pairs; winner column fetched by
dynamic-offset DMA from a replicated fp32 table; gradient, Adam and shrink
run replicated on all cores in fp32.
"""
import math
import numpy as np
import ml_dtypes

import concourse.bass as bass
import concourse.bacc as bacc
import concourse.mybir as mybir
import concourse.tile as tile
from concourse.bass_utils import run_bass_kernel_spmd

F32 = mybir.dt.float32
BF16 = mybir.dt.bfloat16
U32 = mybir.dt.uint32
I32 = mybir.dt.int32
AX = mybir.AxisListType
ALU = mybir.AluOpType
ACT = mybir.ActivationFunctionType

N_CORES = 8
D = 512
K = 16
T = 50000
SHARD = T // N_CORES        # 6250
NCH = 49                    # chunks of 128 T-rows per core
SHPAD = NCH * 128           # 6272
CW = D + K                  # 528: fused winner row (negE | A)
EW = NCH * D                # 25088 bf16 matmul tensor width

LAMBDA1 = 0.3366
LR = 0.1
CSH = LR * LAMBDA1
B1, B2, EPS = 0.9, 0.999, 1e-8
N_ITERS = 10
NEG_BIG = -1.0e30
N_BA_TERMS = 1              # 1: A_hi*B_hi ; 2: + A_hi*B_lo


def build_kernel():
    nc = bacc.Bacc(trn_type="TRN2", num_devices=N_CORES)

    gf_d = nc.dram_tensor("getfull", [T, CW], F32, kind="ExternalInput")
    getbf_d = nc.dram_tensor("getbf", [128, EW], BF16, kind="ExternalInput")
    ahi_d = nc.dram_tensor("a_hi", [128, SHPAD], BF16, kind="ExternalInput")
    rep_d = nc.dram_tensor("repmat", [K, 128], BF16, kind="ExternalInput")
    idb_d = nc.dram_tensor("identbf", [128, 128], BF16, kind="ExternalInput")
    idf_d = nc.dram_tensor("identf", [128, 128], F32, kind="ExternalInput")
    pt_d = nc.dram_tensor("ptinit", [K, D], F32, kind="ExternalInput")
    jb_d = nc.dram_tensor("jbase", [1, 1], F32, kind="ExternalInput")
    pm_d = nc.dram_tensor("padmask", [128, 1], F32, kind="ExternalInput")
    io_d = nc.dram_tensor("iotap", [1, 128], F32, kind="ExternalInput")

    obt_d = nc.dram_tensor("out_bt", [K, D], F32, kind="ExternalOutput")
    dbg_d = nc.dram_tensor("out_dbg", [1, 32], F32, kind="ExternalOutput")

    with tile.TileContext(nc) as tc:
        with tc.tile_pool(name="per", bufs=1) as per, \
             tc.tile_pool(name="scr", bufs=2) as scr, \
             tc.tile_pool(name="pmain", bufs=2, space="PSUM") as pmain, \
             tc.tile_pool(name="ptail", bufs=2, space="PSUM") as ptail, \
             tc.tile_pool(name="dram", bufs=2, space="DRAM") as dram:

            # ---- persistent tiles ----
            getbf = per.tile([128, EW], BF16)
            a_hi = per.tile([128, SHPAD], BF16)
            repm = per.tile([K, 128], BF16)
            BTh4 = per.tile([128, D], BF16)
            identb = per.tile([128, 128], BF16)
            identf = per.tile([128, 128], F32)
            padmask = per.tile([128, 1], F32)
            iotap = per.tile([1, 128], F32)
            jbase = per.tile([1, 1], F32)
            Ct = per.tile([128, 64], F32)
            PT = per.tile([K, D], F32)
            mT = per.tile([K, D], F32)      # m / (1-B1)
            vT = per.tile([K, D], F32)      # v / (1-B2)
            BT = per.tile([K, D], F32)
            BTh = per.tile([K, D], BF16)
            BTl = per.tile([K, D], BF16)
            smult = per.tile([K, D], F32)
            sgnP = per.tile([K, D], F32)
            mask_l1 = per.tile([K, 1], F32)
            dbg = per.tile([1, 32], F32)

            idf = identf[:]

            # ---- loads ----
            NSLAB = 7
            for s in range(NSLAB):
                w = EW // NSLAB
                nc.sync.dma_start(getbf[:, s * w:(s + 1) * w],
                                  getbf_d[:, s * w:(s + 1) * w])
            nc.sync.dma_start(a_hi[:], ahi_d[:])
            nc.sync.dma_start(repm[:], rep_d[:])
            nc.sync.dma_start(identb[:], idb_d[:])
            nc.sync.dma_start(identf[:], idf_d[:])
            nc.sync.dma_start(padmask[:], pm_d[:])
            nc.sync.dma_start(iotap[:], io_d[:])
            nc.sync.dma_start(jbase[:], jb_d[:])
            nc.sync.dma_start(PT[:], pt_d[:])

            nc.vector.memset(mT[:], 0.0)
            nc.vector.memset(vT[:], 0.0)
            nc.vector.memset(Ct[:], NEG_BIG)
            nc.vector.memset(dbg[:], 0.0)
            nc.vector.tensor_copy(BT[:], PT[:])          # iter-1 B = P
            nc.scalar.copy(BTh[:], BT[:])
            rep_ps0 = ptail.tile([128, D], F32, name="rep_ps", tag="tail")
            nc.tensor.matmul(rep_ps0[:], repm[:], BTh[:], start=True, stop=True)
            nc.scalar.copy(BTh4[:], rep_ps0[:])
            if N_BA_TERMS >= 2:
                nc.vector.scalar_tensor_tensor(BTl[:], BTh[:], -1.0, BT[:],
                                               op0=ALU.mult, op1=ALU.add)
            nc.scalar.sign(sgnP[:], PT[:])

            # PE warm-up touches: absorb slab DMA waits one at a time
            for s in range(NSLAB):
                wm = ptail.tile([1, 128], BF16, name="wm", tag="tail")
                nc.tensor.transpose(wm[:], getbf[:, s * (EW // NSLAB):
                                                 s * (EW // NSLAB) + 1],
                                    identb[:])
            gsem = nc.alloc_semaphore("gsem")
            gcnt = 0

            for t in range(1, N_ITERS + 1):
                c2t = 1.0 - B2 ** t
                kt = LR * (1.0 - B1) / (1.0 - B1 ** t)
                sqscale = (1.0 - B2) / c2t

                # ---------- g2 prep: k* = argmax colsum|B| (only needs BT) ----
                cb = scr.tile([K, 1], F32, name="cb", tag="cb")
                nc.vector.tensor_reduce(cb[:], BT[:], axis=AX.X,
                                        op=ALU.add, apply_absolute_value=True)
                cbT = ptail.tile([1, K], F32, name="cbT", tag="tail")
                nc.tensor.transpose(cbT[:], cb[:], idf[0:K, 0:K])
                rcb = scr.tile([1, K], F32, name="rcb", tag="rcb")
                nc.scalar.copy(rcb[:], cbT[:])
                cb8 = scr.tile([1, 8], F32, name="cb8", tag="cb8")
                nc.vector.max(cb8[:], rcb[:])
                kmr = scr.tile([1, K], F32, name="kmr", tag="kmr")
                nc.vector.tensor_scalar(kmr[:], rcb[:], cb8[:, 0:1], None,
                                        op0=ALU.is_ge)
                kmT = ptail.tile([K, 1], F32, name="kmT", tag="tail")
                nc.tensor.transpose(kmT[:], kmr[:], idf[0:1, 0:1])
                nc.scalar.mul(mask_l1[:], kmT[:], LAMBDA1)

                # ---------- main pass: colsums of |B A - E| ----------
                # packs of 3 chunks; BA matmuls run concurrently in PE
                # row-groups 0/32/64 via tile_position; identity-copy matmuls
                # lay -E^T into PSUM first. Reduce split: vector fused
                # abs-reduce for most packs, scalar Abs+accum for the rest.
                NPACK = 17
                N_ACT_PACKS = 6          # packs whose reduce runs on scalar
                for pk in range(NPACK):
                    ng = 3 if pk < NPACK - 1 else 1
                    ps = pmain.tile([128, 1536], F32, name="ps", tag="mainps")
                    for q in range(ng):
                        ch = pk * 3 + q
                        for st in range(4):
                            sl = slice(32 * st, 32 * st + 32)
                            nc.tensor.matmul(
                                ps[sl, q * 512:(q + 1) * 512],
                                identb[sl, sl],
                                getbf[sl, ch * D:(ch + 1) * D],
                                start=True, stop=False,
                                tile_position=(32 * st, 32 * st))
                    for q in range(ng):
                        ch = pk * 3 + q
                        o = ps[:, q * 512:(q + 1) * 512]
                        nc.tensor.matmul(
                            o, a_hi[32 * q:32 * q + K, ch * 128:(ch + 1) * 128],
                            BTh4[32 * q:32 * q + K, :], start=False, stop=True,
                            tile_position=(32 * q, 0))
                    ch0 = pk * 3
                    if NPACK - 1 - N_ACT_PACKS <= pk < NPACK - 1:
                        for q in range(ng):
                            o = ps[:, q * 512:(q + 1) * 512]
                            nc.scalar.activation(o, o, ACT.Abs,
                                                 accum_out=Ct[:, ch0 + q:ch0 + q + 1])
                    else:
                        view = ps[:, 0:ng * 512].rearrange("p (n d) -> p n d",
                                                           d=512)
                        nc.vector.tensor_reduce(Ct[:, ch0: ch0 + ng], view,
                                                axis=AX.X, op=ALU.add,
                                                apply_absolute_value=True)

                # mask pad rows of last chunk
                nc.vector.tensor_scalar(Ct[:, 48:49], Ct[:, 48:49],
                                        padmask[:], None, op0=ALU.add)

                # ---------- local argmax ----------
                m8 = scr.tile([128, 8], F32, name="m8", tag="m8")
                i8 = scr.tile([128, 8], U32, name="i8", tag="i8")
                nc.vector.max(m8[:], Ct[:])
                nc.vector.max_index(i8[:], m8[:], Ct[:])
                ixf = scr.tile([128, 1], F32, name="ixf", tag="ixf")
                nc.vector.tensor_copy(ixf[:], i8[:, 0:1])
                tpm = ptail.tile([1, 128], F32, name="tpm", tag="tail")
                tpi = ptail.tile([1, 128], F32, name="tpi", tag="tail")
                nc.tensor.transpose(tpm[:], m8[:, 0:1], idf)
                nc.tensor.transpose(tpi[:], ixf[:], idf)
                rmx = scr.tile([1, 128], F32, name="rmx", tag="rmx")
                rix = scr.tile([1, 128], F32, name="rix", tag="rix")
                nc.scalar.copy(rmx[:], tpm[:])
                nc.scalar.copy(rix[:], tpi[:])
                g8 = scr.tile([1, 8], F32, name="g8", tag="g8")
                gi8 = scr.tile([1, 8], U32, name="gi8", tag="gi8")
                nc.vector.max(g8[:], rmx[:])
                nc.vector.max_index(gi8[:], g8[:], rmx[:])
                ploc = scr.tile([1, 1], F32, name="ploc", tag="ploc")
                nc.vector.tensor_copy(ploc[:], gi8[:, 0:1])
                # cloc = rix[ploc]
                eqm = scr.tile([1, 128], F32, name="eqm", tag="eqm")
                nc.vector.tensor_scalar(eqm[:], iotap[:], ploc[:], None,
                                        op0=ALU.is_equal)
                nc.vector.tensor_tensor(out=eqm[:], in0=eqm[:], in1=rix[:],
                                        op=ALU.mult)
                cloc = scr.tile([1, 1], F32, name="cloc", tag="cloc")
                nc.vector.tensor_reduce(cloc[:], eqm[:], axis=AX.X, op=ALU.add)
                # jglob = jbase + cloc*128 + ploc
                jg = scr.tile([1, 1], F32, name="jg", tag="jg")
                nc.vector.scalar_tensor_tensor(jg[:], cloc[:], 128.0, ploc[:],
                                               op0=ALU.mult, op1=ALU.add)
                nc.vector.tensor_tensor(out=jg[:], in0=jg[:], in1=jbase[:],
                                        op=ALU.add)

                # ---------- AllGather of (maxval, jglob) ----------
                blob = scr.tile([1, 8], F32, name="blob", tag="blob")
                nc.scalar.copy(blob[:, 0:1], g8[:, 0:1])
                nc.scalar.copy(blob[:, 1:2], jg[:])
                agi = dram.tile([1, 8], F32, name="agi", tag="agi")
                ago = dram.tile([8, 8], F32, name="ago", tag="ago",
                                addr_space="Shared")
                nc.sync.dma_start(agi[:], blob[:])
                nc.gpsimd.collective_compute(
                    "AllGather", ALU.bypass,
                    replica_groups=[list(range(N_CORES))],
                    ins=[agi[:]], outs=[ago[:]])

                # ---------- winner ----------
                vals8 = scr.tile([1, 8], F32, name="vals8", tag="vals8")
                jg8 = scr.tile([1, 8], F32, name="jg8", tag="jg8")
                with nc.allow_non_contiguous_dma(reason="8-elem gathers"):
                    nc.sync.dma_start(vals8[:], ago[:, 0:1].transpose([1, 0]))
                    nc.sync.dma_start(jg8[:], ago[:, 1:2].transpose([1, 0]))
                w8 = scr.tile([1, 8], F32, name="w8", tag="w8")
                wi8 = scr.tile([1, 8], U32, name="wi8", tag="wi8")
                nc.vector.max(w8[:], vals8[:])
                nc.vector.max_index(wi8[:], w8[:], vals8[:])
                wf = scr.tile([1, 1], F32, name="wf", tag="wf")
                nc.vector.tensor_copy(wf[:], wi8[:, 0:1])
                eqw = scr.tile([1, 8], F32, name="eqw", tag="eqw")
                nc.vector.tensor_scalar(eqw[:], iotap[:, 0:8], wf[:], None,
                                        op0=ALU.is_equal)
                nc.vector.tensor_tensor(out=eqw[:], in0=eqw[:], in1=jg8[:],
                                        op=ALU.mult)
                jwin = scr.tile([1, 1], F32, name="jwin", tag="jwin")
                nc.vector.tensor_reduce(jwin[:], eqw[:], axis=AX.X, op=ALU.add)
                jwi = scr.tile([1, 1], I32, name="jwi", tag="jwi")
                nc.vector.tensor_copy(jwi[:], jwin[:])
                winner = scr.tile([1, CW], F32, name="winner", tag="winner")
                with tc.tile_critical():
                    jv = nc.gpsimd.value_load(jwi[:])
                    gcnt += 16
                    nc.gpsimd.dma_start(
                        winner[:], gf_d[bass.ds(jv, 1), :]).then_inc(gsem, 16)
                    nc.gpsimd.wait_ge(gsem, gcnt)

                # debug capture
                nc.scalar.copy(dbg[:, t - 1:t], jwin[:])
                nc.scalar.copy(dbg[:, 9 + t:10 + t], w8[:, 0:1])

                # ---------- gradient (fp32 exact path) ----------
                acT = ptail.tile([K, 1], F32, name="acT", tag="tail")
                nc.tensor.transpose(acT[:], winner[:, D:D + K], idf[0:1, 0:1])
                acol = scr.tile([K, 1], F32, name="acol", tag="acol")
                nc.scalar.copy(acol[:], acT[:])
                ba = ptail.tile([1, D], F32, name="ba", tag="tail")
                nc.tensor.matmul(ba[:], acol[:], BT[:], start=True, stop=True)
                u = scr.tile([1, D], F32, name="u", tag="u")
                nc.vector.tensor_tensor(out=u[:], in0=winner[:, 0:D],
                                        in1=ba[:], op=ALU.add)
                srow = scr.tile([1, D], F32, name="srow", tag="srow")
                nc.scalar.sign(srow[:], u[:])
                g1T = ptail.tile([K, D], F32, name="g1T", tag="tail")
                nc.tensor.matmul(g1T[:], winner[:, D:D + K], srow[:],
                                 start=True, stop=True)

                gT = scr.tile([K, D], F32, name="gT", tag="gT")
                nc.vector.scalar_tensor_tensor(gT[:], sgnP[:], mask_l1[:],
                                               g1T[:], op0=ALU.mult, op1=ALU.add)
                if t >= 2:
                    nc.vector.tensor_tensor(out=gT[:], in0=gT[:], in1=smult[:],
                                            op=ALU.mult)

                # ---------- Adam (rescaled moments; eps folded away) --------
                sq = scr.tile([K, D], F32, name="sq", tag="sq")
                nc.scalar.square(sq[:], gT[:])
                nc.vector.scalar_tensor_tensor(vT[:], vT[:], B2, sq[:],
                                               op0=ALU.mult, op1=ALU.add)
                nc.vector.scalar_tensor_tensor(mT[:], mT[:], B1, gT[:],
                                               op0=ALU.mult, op1=ALU.add)
                dn = scr.tile([K, D], F32, name="dn", tag="dn")
                nc.scalar.activation(dn[:], vT[:], ACT.Sqrt, scale=sqscale)
                rcp = scr.tile([K, D], F32, name="rcp", tag="rcp")
                rscr = scr.tile([K, D], F32, name="rscr", tag="rscr")
                nc.vector.reciprocal_approx_accurate(rcp[:], dn[:], rscr[:])
                nc.vector.tensor_tensor(out=rcp[:], in0=mT[:], in1=rcp[:],
                                        op=ALU.mult)
                nc.vector.scalar_tensor_tensor(PT[:], rcp[:], -kt, PT[:],
                                               op0=ALU.mult, op1=ALU.add)

                # ---------- shrink / next-iteration B ----------
                t1 = scr.tile([K, D], F32, name="t1", tag="t1")
                nc.vector.tensor_scalar_sub(t1[:], PT[:], CSH)
                nc.scalar.sign(sgnP[:], PT[:])
                ab1 = scr.tile([K, D], F32, name="ab1", tag="ab1")
                nc.scalar.activation(ab1[:], t1[:], ACT.Abs)
                if t < N_ITERS:
                    nc.vector.tensor_tensor(out=BTh[:], in0=sgnP[:], in1=ab1[:],
                                            op=ALU.mult)
                    rep_ps = ptail.tile([128, D], F32, name="rep_ps", tag="tail")
                    nc.tensor.matmul(rep_ps[:], repm[:], BTh[:],
                                     start=True, stop=True)
                    nc.scalar.copy(BTh4[:], rep_ps[:])
                    nc.vector.tensor_tensor(out=BT[:], in0=sgnP[:], in1=ab1[:],
                                            op=ALU.mult)
                    if N_BA_TERMS >= 2:
                        nc.vector.scalar_tensor_tensor(
                            BTl[:], BTh[:], -1.0, BT[:],
                            op0=ALU.mult, op1=ALU.add)
                    sg1 = scr.tile([K, D], F32, name="sg1", tag="sg1")
                    nc.scalar.sign(sg1[:], t1[:])
                    nc.vector.tensor_tensor(out=smult[:], in0=sgnP[:],
                                            in1=sg1[:], op=ALU.mult)
                else:
                    nc.vector.tensor_tensor(out=BT[:], in0=sgnP[:], in1=ab1[:],
                                            op=ALU.mult)
                    nc.sync.dma_start(obt_d[:], BT[:])
                    nc.sync.dma_start(dbg_d[:], dbg[:])

    nc.compile()
    return nc


_CACHE = {}


def _prep_inputs(embedding, basis_init, activation_init):
    E = np.ascontiguousarray(embedding, dtype=np.float32)
    A = np.ascontiguousarray(activation_init, dtype=np.float32)
    B0 = np.ascontiguousarray(basis_init, dtype=np.float32)
    ET = np.ascontiguousarray(E.T)              # (T, D)
    identf = np.eye(128, dtype=np.float32)
    identb = identf.astype(ml_dtypes.bfloat16)
    ptinit = np.ascontiguousarray(B0.T)         # (K, D)
    padmask = np.zeros((128, 1), np.float32)
    padmask[SHARD - 48 * 128:, :] = NEG_BIG
    iotap = np.arange(128, dtype=np.float32).reshape(1, 128)

    getfull = np.ascontiguousarray(
        np.concatenate([-ET, A.T], axis=1))     # (T, CW) shared by all cores
    repmat = np.zeros((K, 128), np.float32)
    for qq in range(4):
        repmat[np.arange(K), 32 * qq + np.arange(K)] = 1.0
    repmat = repmat.astype(ml_dtypes.bfloat16)

    in_maps = []
    for c in range(N_CORES):
        lo = c * SHARD
        slabE = np.zeros((SHPAD, D), np.float32)
        slabE[:SHARD] = -ET[lo:lo + SHARD]
        getbf = np.ascontiguousarray(
            slabE.reshape(NCH, 128, D).transpose(1, 0, 2).reshape(128, EW)
        ).astype(ml_dtypes.bfloat16)
        ash = np.zeros((128, SHPAD), np.float32)
        for qq in range(3):
            ash[32 * qq:32 * qq + K, :SHARD] = A[:, lo:lo + SHARD]
        in_maps.append({
            "getfull": getfull,
            "getbf": getbf,
            "a_hi": ash.astype(ml_dtypes.bfloat16),
            "repmat": repmat,
            "identbf": identb,
            "identf": identf,
            "ptinit": ptinit,
            "jbase": np.array([[float(lo)]], np.float32),
            "padmask": padmask,
            "iotap": iotap,
        })
    return in_maps


def kernel(embedding, basis_init, activation_init, k, _want_debug=False):
    if "nc" not in _CACHE:
        _CACHE["nc"] = build_kernel()
    nc = _CACHE["nc"]
    in_maps = _prep_inputs(embedding, basis_init, activation_init)
    res = run_bass_kernel_spmd(nc, in_maps, core_ids=list(range(N_CORES)))
    r0 = res.results[0]
    out_b = np.ascontiguousarray(r0["out_bt"].T)      # (D, K)
    out_a = np.asarray(activation_init, dtype=np.float32)
    _CACHE["last_res"] = res
    _CACHE["last_dbg"] = r0["out_dbg"]
    if _want_debug:
        return (out_b, out_a), r0["out_dbg"]
    return out_b, out_a
